# revision 1
# baseline (speedup 1.0000x reference)
"""Trainium2 Bass kernel for per-token outer-product softmax attention.

Reference computation (per token t of 1600, H=256):
    k = tanh(x W0 + b0);  q = tanh(x W1 + b1)
    scores[i,j] = k[i]*q[j];  attn = softmax_j(scores);  out = attn @ x

Key algebra: k,q are tanh outputs so k[i]*q[j] in (-1,1). On [-1,1],
exp(s) is approximated to fp32-noise level by a low-degree polynomial
P(s) = sum_d c_d s^d, and P(k_i q_j) = sum_d c_d k_i^d q_j^d is
SEPARABLE. Softmax numerator/denominator become per-token moments:
    num_i = sum_d (c_d sum_j q_j^d x_j) k_i^d
    den_i = sum_d (c_d sum_j q_j^d)     k_i^d
so the 256x256 scores tensor is never materialized. Per 128-token tile
this is ~2D fused multiply+reduce passes (moments, via
scalar_tensor_tensor accum_out) plus two fused Horner chains over k,
all [128,256] vector instructions spread across DVE / GpSimd(Pool) /
ACT engines. The queries matmul+tanh is scheduled before the keys one
so the moment pipeline starts ASAP; the final +a0 of the numerator
chain is fused with the divide.

Sharding: pure data parallel over tokens, 200 tokens/core x 8 cores;
weights replicated.
"""

import numpy as np
from contextlib import ExitStack

import concourse.bass as bass
import concourse.bacc as bacc
import concourse.tile as tile
from concourse import mybir
from concourse.bass_utils import run_bass_kernel_spmd

F32 = mybir.dt.float32
AF = mybir.ActivationFunctionType
OP = mybir.AluOpType

B, S, M, H = 4, 10, 40, 256
T = B * S * M            # 1600 tokens
NCORES = 8
TC = T // NCORES         # 200 tokens per core
BLOCKS = [(0, 128), (128, TC - 128)]

# Chebyshev-interpolation coefficients (monomial basis) of exp on [-1,1].
# Max rel err: D=6 -> 7.7e-6, D=8 -> 2.7e-8.
COEFS = {
    6: [1.0, 1.000022235, 0.5000027659, 0.1664890938, 0.04164456983,
        0.008686644402, 0.001432899535],
    8: [1.0, 0.9999999011, 0.4999999901, 0.1666679842, 0.04166679799,
        0.008328598904, 0.001388416857, 0.0002046983349, 2.542872193e-05],
}

D = 6

# Engine assignment knobs (tuned against real-HW loop benchmarks):
CFG = {
    "n_den_act": 6,     # denominator accums d=2..D: first n on ACT, rest DVE TS+accum
    "n_num_pool": 0,    # numerator moments d=2..D: first n via Pool TT + ACT accum
    "chain_tt_pool": 3,  # estrin only: of the 12 combine-TTs, how many on Pool
    "pairs_act": 8,     # estrin only: of the 8 pairs per block, how many on ACT
    "j0_act": True,     # d=0 numerator moment on ACT instead of DVE
    "tree_dve": 0,      # of the QP-tree TTs, how many on DVE instead of Pool
    "kpow_dve": 0,      # estrin only: of the 3 K-power TTs, how many on DVE
    "x_dma": "sync",    # engine for X loads: sync | scalar | gpsimd
    "out_dma": "sync",  # engine for output stores
    "recip": "approx",  # approx (~2 ULP custom DVE) | exact
    "scrp_bufs": 8,
    "phase_limit": 4,   # 0=min body, 1=KQ only, 2=+moments, 3=+chains, 4=full
    "chain_mode": "horner_dve",  # estrin | horner_dve | horner_mix
}


def _pow_tree(dmax):
    """Return list of (d, a, b) meaning QP_d = QP_a * QP_b, log-depth order."""
    steps = []
    have = {1}
    for d in range(2, dmax + 1):
        a = d // 2
        b = d - a
        steps.append((d, a, b))
        have.add(d)
    return steps


def build_kernel(reps: int = 1, with_bias: bool = True) -> bass.Bass:
    coef = COEFS[D]
    # wcat columns: [W1lo|W1hi|biasQ|coef || W0lo|W0hi|biasK]
    WQ = 2 * H + H + 2 * (D + 1)   # 786
    WK = 2 * H + H                 # 768
    WEXT = WQ + WK
    nc = bacc.Bacc("TRN2", target_bir_lowering=False, debug=False)
    xs = nc.declare_dram_parameter("xs", [TC, H], F32, isOutput=False)
    xst = nc.declare_dram_parameter("xst", [128, 2, TC], F32, isOutput=False)
    wcat = nc.declare_dram_parameter("wcat", [128, WEXT], F32, isOutput=False)
    out = nc.declare_dram_parameter("out", [TC, H], F32, isOutput=True)

    with tile.TileContext(nc) as tc, ExitStack() as ctx:
        consts = ctx.enter_context(tc.tile_pool(name="consts", bufs=1))
        io = ctx.enter_context(tc.tile_pool(name="io", bufs=CFG.get("io_bufs", 2)))
        work = ctx.enter_context(tc.tile_pool(name="work", bufs=CFG.get("work_bufs", 2)))
        pows = ctx.enter_context(tc.tile_pool(name="pows", bufs=CFG.get("pows_bufs", 2)))
        scrp = ctx.enter_context(tc.tile_pool(name="scrp", bufs=CFG.get("scrp_bufs", 3)))
        mom = ctx.enter_context(tc.tile_pool(name="mom", bufs=2))
        psKQ = ctx.enter_context(
            tc.tile_pool(name="psKQ", bufs=CFG.get("pskq_bufs", 2), space="PSUM")
        )

        x_eng = getattr(nc, CFG["x_dma"])
        out_eng = getattr(nc, CFG["out_dma"])
        # Small constants first on the Pool queue, then X (gates the whole
        # pipeline), then the Q-side weights (gate MM-Q), then K-side.
        ones1 = consts.tile([1, 128], F32)
        nc.gpsimd.memset(ones1, 1.0)
        Xs = []
        XTs = []
        for t0, tl in BLOCKS:
            X = io.tile([128, H], F32, tag=f"X{t0}")
            x_eng.dma_start(out=X[:tl, :], in_=xs[t0 : t0 + tl, :])
            Xs.append(X)
            xT = io.tile([128, 2, 128], F32, tag=f"XT{t0}")
            # gpsimd queue: runs in parallel with the X loads on sync HWDGE
            nc.gpsimd.dma_start(out=xT[:, :, :tl], in_=xst[:, :, t0 : t0 + tl])
            XTs.append(xT)
        wallQ = consts.tile([128, WQ], F32)
        nc.gpsimd.dma_start(out=wallQ, in_=wcat[:, 0:WQ])
        wallK = consts.tile([128, WK], F32)
        nc.gpsimd.dma_start(out=wallK, in_=wcat[:, WQ:WEXT])
        bsbQ = wallQ[0:1, 2 * H : 3 * H]
        bsbK = wallK[0:1, 2 * H : 3 * H]
        ctile = wallQ[:, 3 * H : 3 * H + 2 * (D + 1)].rearrange(
            "p (two d) -> p two d", two=2
        )

        def body():
            if CFG["phase_limit"] == 0:
                for t0, tl in BLOCKS:
                    O = io.tile([128, H], F32, tag="O")
                    nc.vector.tensor_copy(O[:tl, :], Xs[0][:tl, :])
                    out_eng.dma_start(out=out[t0 : t0 + tl, :], in_=O[:tl, :])
                return
            for bi, (t0, tl) in enumerate(BLOCKS):
                X = Xs[bi]
                xT = XTs[bi]  # x^T pre-transposed on host

                # ---- queries first: moments only need Q and X.
                # Bias matmul leads: it only needs constants, so it runs
                # during the xT dependency chain.
                psQ = psKQ.tile([128, H], F32, tag="psQ")
                if with_bias:
                    nc.tensor.matmul(
                        psQ[:tl, :], ones1[:, :tl], bsbQ,
                        start=True, stop=False,
                    )
                nc.tensor.matmul(
                    psQ[:tl, :], xT[:, 0, :tl], wallQ[:, 0:256],
                    start=not with_bias, stop=False,
                )
                nc.tensor.matmul(
                    psQ[:tl, :], xT[:, 1, :tl], wallQ[:, 256:512],
                    start=False, stop=True,
                )
                # Smom[:, 0, :] = raw numerator moments, [:, 1, :] = denominator
                Smom = mom.tile([128, 2, D + 1], F32, tag="Smom")
                nc.gpsimd.memset(Smom[:tl, 1, 0:1], float(H))
                Qt = work.tile([128, H], F32, tag="Qt")
                nc.scalar.activation(
                    Qt[:tl, :], psQ[:tl, :], AF.Tanh,
                    accum_out=Smom[:tl, 1, 1:2],
                )
                Q = Qt[:tl, :]

                # ---- keys (overlaps with the moment pipeline below)
                psK = psKQ.tile([128, H], F32, tag="psK")
                if with_bias:
                    nc.tensor.matmul(
                        psK[:tl, :], ones1[:, :tl], bsbK,
                        start=True, stop=False,
                    )
                nc.tensor.matmul(
                    psK[:tl, :], xT[:, 0, :tl], wallK[:, 0:256],
                    start=not with_bias, stop=False,
                )
                nc.tensor.matmul(
                    psK[:tl, :], xT[:, 1, :tl], wallK[:, 256:512],
                    start=False, stop=True,
                )
                Kt = work.tile([128, H], F32, tag="Kt")
                nc.scalar.activation(Kt[:tl, :], psK[:tl, :], AF.Tanh)
                K = Kt[:tl, :]

                if CFG["phase_limit"] == 1:
                    O = io.tile([128, H], F32, tag="O")
                    nc.vector.tensor_add(O[:tl, :], Qt[:tl, :], Kt[:tl, :])
                    out_eng.dma_start(out=out[t0 : t0 + tl, :], in_=O[:tl, :])
                    continue

                # ---- raw moments (unscaled powers QP_d = q^d)
                j0 = scrp.tile([128, H], F32, tag="scr")
                if CFG["j0_act"]:
                    nc.scalar.activation(
                        j0[:tl, :], X[:tl, :], AF.Identity,
                        accum_out=Smom[:tl, 0, 0:1],
                    )
                else:
                    nc.vector.tensor_scalar(
                        out=j0[:tl, :], in0=X[:tl, :], scalar1=1.0, scalar2=0.0,
                        op0=OP.mult, op1=OP.add, accum_out=Smom[:tl, 0, 0:1],
                    )
                s1 = scrp.tile([128, H], F32, tag="scr")
                nc.vector.scalar_tensor_tensor(
                    out=s1[:tl, :], in0=Q, scalar=1.0, in1=X[:tl, :],
                    op0=OP.mult, op1=OP.mult, accum_out=Smom[:tl, 0, 1:2],
                )
                QP = {1: Q}
                n_act = 0
                n_pool = 0
                n_tree_dve = 0
                for d, a, b in _pow_tree(D):
                    QPn = pows.tile([128, H], F32, tag=f"qp{d}")
                    if n_tree_dve < CFG["tree_dve"]:
                        n_tree_dve += 1
                        nc.vector.tensor_mul(QPn[:tl, :], QP[a], QP[b])
                    else:
                        nc.gpsimd.tensor_mul(QPn[:tl, :], QP[a], QP[b])
                    QP[d] = QPn[:tl, :]
                    # denominator accum
                    if n_act < CFG["n_den_act"]:
                        n_act += 1
                        ja = scrp.tile([128, H], F32, tag="scr")
                        nc.scalar.activation(
                            ja[:tl, :], QPn[:tl, :], AF.Identity,
                            accum_out=Smom[:tl, 1, d : d + 1],
                        )
                    elif CFG.get("den_dve_op", "ts") == "ts":
                        jr = scrp.tile([128, H], F32, tag="scr")
                        nc.vector.tensor_scalar(
                            out=jr[:tl, :], in0=QPn[:tl, :], scalar1=1.0,
                            scalar2=0.0, op0=OP.mult, op1=OP.add,
                            accum_out=Smom[:tl, 1, d : d + 1],
                        )
                    else:
                        nc.vector.tensor_reduce(
                            out=Smom[:tl, 1, d : d + 1], in_=QPn[:tl, :],
                            axis=mybir.AxisListType.X, op=OP.add,
                        )
                    # numerator moment: sum (q^d * x)
                    if n_pool < CFG["n_num_pool"]:
                        n_pool += 1
                        sd = scrp.tile([128, H], F32, tag="scr")
                        nc.gpsimd.tensor_mul(sd[:tl, :], QPn[:tl, :], X[:tl, :])
                        jb = scrp.tile([128, H], F32, tag="scr")
                        nc.scalar.activation(
                            jb[:tl, :], sd[:tl, :], AF.Identity,
                            accum_out=Smom[:tl, 0, d : d + 1],
                        )
                    else:
                        sd = scrp.tile([128, H], F32, tag="scr")
                        nc.vector.scalar_tensor_tensor(
                            out=sd[:tl, :], in0=QPn[:tl, :], scalar=1.0,
                            in1=X[:tl, :], op0=OP.mult, op1=OP.mult,
                            accum_out=Smom[:tl, 0, d : d + 1],
                        )

                # ---- scale moments by polynomial coefficients (one tiny TT)
                A2 = mom.tile([128, 2, D + 1], F32, tag="A2")
                nc.vector.tensor_mul(A2[:tl, :, :], Smom[:tl, :, :], ctile[:tl, :, :])

                if CFG["phase_limit"] == 2:
                    O = io.tile([128, H], F32, tag="O")
                    nc.vector.tensor_copy(O[:tl, :], K)
                    nc.vector.tensor_scalar(
                        out=O[:tl, 0 : 2 * (D + 1)],
                        in0=A2[:tl, :, :].rearrange("p a b -> p (a b)"),
                        scalar1=1.0, scalar2=None, op0=OP.mult,
                    )
                    out_eng.dma_start(out=out[t0 : t0 + tl, :], in_=O[:tl, :])
                    continue

                # ---- K powers for Estrin: k^2, k^4, k^8
                if CFG["chain_mode"] == "estrin":
                    kp_engs = [nc.vector] * CFG["kpow_dve"] + [nc.gpsimd] * 3
                    K2 = pows.tile([128, H], F32, tag="K2")
                    kp_engs[0].tensor_mul(K2[:tl, :], K, K)
                    K4 = pows.tile([128, H], F32, tag="K4")
                    kp_engs[1].tensor_mul(K4[:tl, :], K2[:tl, :], K2[:tl, :])
                    K8 = pows.tile([128, H], F32, tag="K8")
                    kp_engs[2].tensor_mul(K8[:tl, :], K4[:tl, :], K4[:tl, :])

                # ---- Estrin evaluation of both polynomials over K
                # P(k) = (a0 + a1 k) + k^2 (a2 + a3 k)
                #      + k^4 [(a4 + a5 k) + k^2 (a6 + a7 k)] + a8 k^8
                cnt = {"pair": 0, "tt": 0}

                def estrin(which, tag):
                    a = lambda d: A2[:tl, which, d : d + 1]
                    ps = []
                    for i in range(4):
                        p = scrp.tile([128, H], F32, tag=f"p{tag}{i}")
                        if cnt["pair"] < CFG["pairs_act"]:
                            cnt["pair"] += 1
                            nc.scalar.activation(
                                p[:tl, :], K, AF.Identity,
                                scale=a(2 * i + 1), bias=a(2 * i),
                            )
                        else:
                            nc.vector.tensor_scalar(
                                out=p[:tl, :], in0=K, scalar1=a(2 * i + 1),
                                scalar2=a(2 * i), op0=OP.mult, op1=OP.add,
                            )
                        ps.append(p)
                    n_pool_tt = CFG["chain_tt_pool"]
                    engs = []
                    for _ in range(6):
                        engs.append(
                            nc.gpsimd if cnt["tt"] < n_pool_tt else nc.vector
                        )
                        cnt["tt"] += 1
                    t1 = scrp.tile([128, H], F32, tag=f"t1{tag}")
                    engs[0].tensor_mul(t1[:tl, :], ps[1][:tl, :], K2[:tl, :])
                    e01 = scrp.tile([128, H], F32, tag=f"e01{tag}")
                    engs[1].tensor_add(e01[:tl, :], t1[:tl, :], ps[0][:tl, :])
                    t2 = scrp.tile([128, H], F32, tag=f"t2{tag}")
                    engs[2].tensor_mul(t2[:tl, :], ps[3][:tl, :], K2[:tl, :])
                    e23 = scrp.tile([128, H], F32, tag=f"e23{tag}")
                    engs[3].tensor_add(e23[:tl, :], t2[:tl, :], ps[2][:tl, :])
                    t3 = scrp.tile([128, H], F32, tag=f"t3{tag}")
                    engs[4].tensor_mul(t3[:tl, :], e23[:tl, :], K4[:tl, :])
                    f = scrp.tile([128, H], F32, tag=f"f{tag}")
                    engs[5].tensor_add(f[:tl, :], t3[:tl, :], e01[:tl, :])
                    res = work.tile([128, H], F32, tag=f"res{tag}")
                    nc.vector.scalar_tensor_tensor(
                        out=res[:tl, :], in0=K8[:tl, :], scalar=a(8),
                        in1=f[:tl, :], op0=OP.mult, op1=OP.add,
                    )
                    return res

                def horner_chain(which, tag, add_eng, mul_eng, skip_final=False):
                    # u = a_D k; repeat: u = (u + a_d) * k; final +a_0
                    a = lambda d: A2[:tl, which, d : d + 1]
                    u = work.tile([128, H], F32, tag=f"res{tag}")
                    nc.vector.tensor_scalar(
                        out=u[:tl, :], in0=K, scalar1=a(D), scalar2=None,
                        op0=OP.mult,
                    )
                    for d in range(D - 1, 0, -1):
                        if add_eng is None:
                            nc.vector.scalar_tensor_tensor(
                                out=u[:tl, :], in0=u[:tl, :], scalar=a(d),
                                in1=K, op0=OP.add, op1=OP.mult,
                            )
                        else:
                            add_eng(u, a(d))
                            mul_eng.tensor_mul(u[:tl, :], u[:tl, :], K)
                    if not skip_final:
                        nc.vector.tensor_scalar(
                            out=u[:tl, :], in0=u[:tl, :], scalar1=a(0),
                            scalar2=None, op0=OP.add,
                        )
                    return u

                mode = CFG["chain_mode"]
                skip_a0 = {"skip": False}
                if mode == "estrin":
                    uN = estrin(0, "n")
                    uD = estrin(1, "d")
                elif mode == "horner_dve":
                    skip_a0["skip"] = True
                    uN = horner_chain(0, "n", None, None, skip_final=True)
                    uD = horner_chain(1, "d", None, None)
                else:  # horner_mix: numerator on DVE, denominator ACT/Pool
                    uN = horner_chain(0, "n", None, None)

                    def act_add(u, aap):
                        nc.scalar.activation(
                            out=u[:tl, :], in_=u[:tl, :], func=AF.Identity,
                            bias=aap,
                        )

                    uD = horner_chain(1, "d", act_add, nc.gpsimd)

                if CFG["phase_limit"] == 3:
                    O = io.tile([128, H], F32, tag="O")
                    nc.vector.tensor_add(O[:tl, :], uN[:tl, :], uD[:tl, :])
                    out_eng.dma_start(out=out[t0 : t0 + tl, :], in_=O[:tl, :])
                    continue

                # ---- out = num / den
                rD = work.tile([128, H], F32, tag="rD")
                if CFG["recip"] == "fast":
                    nc.vector.reciprocal_approx_fast(rD[:tl, :], uD[:tl, :])
                elif CFG["recip"] == "approx":
                    rs = scrp.tile([128, H], F32, tag="scr")
                    nc.vector.reciprocal_approx_accurate(
                        rD[:tl, :], uD[:tl, :], rs[:tl, :]
                    )
                else:
                    nc.vector.reciprocal(rD[:tl, :], uD[:tl, :])
                O = io.tile([128, H], F32, tag="O")
                if skip_a0["skip"]:
                    # fused: out = (uN + a0_num) * (1/den)
                    nc.vector.scalar_tensor_tensor(
                        out=O[:tl, :], in0=uN[:tl, :],
                        scalar=A2[:tl, 0, 0:1], in1=rD[:tl, :],
                        op0=OP.add, op1=OP.mult,
                    )
                else:
                    fm_eng = nc.vector if CFG.get("fmul_dve") else nc.gpsimd
                    fm_eng.tensor_mul(O[:tl, :], uN[:tl, :], rD[:tl, :])
                out_eng.dma_start(out=out[t0 : t0 + tl, :], in_=O[:tl, :])

        if reps == 1:
            body()
        else:
            with tc.For_i(0, reps, 1):
                body()

    nc.compile()
    return nc


_NCS = {}


def _get_nc(with_bias: bool = True):
    if with_bias not in _NCS:
        _NCS[with_bias] = build_kernel(with_bias=with_bias)
    return _NCS[with_bias]


def _make_in_maps(x, W0, b0, W1, b1):
    coef = COEFS[D]
    xf = np.ascontiguousarray(np.asarray(x, np.float32).reshape(T, H))
    W0 = np.asarray(W0, np.float32)
    W1 = np.asarray(W1, np.float32)
    biasQ = np.zeros((128, H), np.float32)
    biasQ[0, :] = np.asarray(b1, np.float32)
    biasK = np.zeros((128, H), np.float32)
    biasK[0, :] = np.asarray(b0, np.float32)
    c2 = np.tile(
        np.array(coef + coef, np.float32).reshape(1, 2 * (D + 1)), (128, 1)
    )
    wcat = np.ascontiguousarray(
        np.concatenate(
            [W1[:128, :], W1[128:, :], biasQ, c2,
             W0[:128, :], W0[128:, :], biasK],
            axis=1,
        )
    )  # [128, WQ+WK]
    maps = []
    for c in range(NCORES):
        sh = np.ascontiguousarray(xf[c * TC : (c + 1) * TC])  # [TC, H]
        # xst[h, chunk, t] = sh[t, chunk*128 + h]
        xst = np.ascontiguousarray(
            np.transpose(sh.reshape(TC, 2, 128), (2, 1, 0))
        )
        maps.append({"xs": sh, "xst": xst, "wcat": wcat})
    return maps


def _ensure_axon():
    # The PJRT path needs the axon devices as jax's default platform; if a
    # caller pinned cpu before importing us, try to restore axon.
    try:
        import jax
        if not any(d.platform == "axon" for d in jax.devices()):
            jax.config.update("jax_platforms", "axon,cpu")
    except Exception:
        pass


def _run(x, W0, b0, W1, b1, trace=False, **kw):
    _ensure_axon()
    with_bias = bool(
        np.any(np.asarray(b0, np.float32)) or np.any(np.asarray(b1, np.float32))
    )
    res = run_bass_kernel_spmd(
        _get_nc(with_bias), _make_in_maps(x, W0, b0, W1, b1),
        list(range(NCORES)), trace=trace, **kw,
    )
    outs = [res.results[c]["out"] for c in range(NCORES)]
    full = np.concatenate(outs, axis=0).reshape(B, S, M, H).astype(np.float32)
    return full, res


def kernel(x, W0, b0, W1, b1):
    full, _ = _run(x, W0, b0, W1, b1, trace=False)
    return full



# revision 7
# speedup vs baseline: 1.0984x; 1.0984x over previous
"""Trainium2 Bass kernel for per-token outer-product softmax attention.

Reference computation (per token t of 1600, H=256):
    k = tanh(x W0 + b0);  q = tanh(x W1 + b1)
    scores[i,j] = k[i]*q[j];  attn = softmax_j(scores);  out = attn @ x

Key algebra: k,q are tanh outputs so k[i]*q[j] in (-1,1). On [-1,1],
exp(s) is approximated to well below the 2e-2 output tolerance by a
low-degree minimax polynomial P(s) = sum_d c_d s^d, and P(k_i q_j) =
sum_d c_d k_i^d q_j^d is SEPARABLE. Softmax numerator/denominator become
per-token moments:
    num_i = sum_d (c_d sum_j q_j^d x_j) k_i^d
    den_i = sum_d (c_d sum_j q_j^d)     k_i^d
so the 256x256 scores tensor is never materialized. Moments come free as
accum_out of the product ops (coefficients folded into the op scalars);
the polynomial in k is evaluated in the power basis (powers via ACT
Square + one tensor-mul) so both chains are short independent
multiply-accumulate sequences that can be split across DVE and Pool.

Sharding: pure data parallel over tokens, 200 tokens/core x 8 cores;
weights replicated.
"""

import numpy as np
from contextlib import ExitStack

import concourse.bass as bass
import concourse.bacc as bacc
import concourse.tile as tile
from concourse import mybir
from concourse.bass_utils import run_bass_kernel_spmd

F32 = mybir.dt.float32
AF = mybir.ActivationFunctionType
OP = mybir.AluOpType

B, S, M, H = 4, 10, 40, 256
T = B * S * M            # 1600 tokens
NCORES = 8
TC = T // NCORES         # 200 tokens per core
BLOCKS = [(0, 128), (128, TC - 128)]

# Minimax-relative-error coefficients (monomial basis) of exp on [-1,1].
# Poly max rel err: D=3 -> 5.0e-3, D=4 -> 5.0e-4; end-to-end output
# rel-L2 err (fp32): D=3 -> 2.9e-3, D=4 -> 2.9e-4 (tolerance 2e-2).
COEFS = {
    3: [0.99650635, 1.0107962638, 0.5388581246, 0.1585305384],
    4: [0.9996280079, 0.9979377479, 0.5028966853, 0.1764876527,
        0.0399652955],
    6: [1.0, 1.000022235, 0.5000027659, 0.1664890938, 0.04164456983,
        0.008686644402, 0.001432899535],
}

D = 3

# Per-op engine placement (tuned on HW loop benchmarks).
# "dve" = VectorE, "pool" = GpSimd, "act" = ScalarE (limited ops).
# NOTE: TensorScalarPtr (tensor_scalar / scalar_tensor_tensor) is NOT
# legal on Pool — Pool gets only plain tensor_tensor products.
CFG = {
    "m0": "act",               # aN0 accum: Identity/TS on X, scale=c0
    # m1..mD: "dve" = one STT; "pool_dve" = Pool TT product + DVE
    # TS-accum; "pool_act" = Pool TT product + ACT Identity-accum.
    "m": ["dve", "pool_dve", "pool_dve", "dve"][: D],
    "s3": "act",               # s3 scaled accum: "act" | "dve"
    "qpow3": "pool",           # Q3 = Q2*Q tensor-mul
    "kpow3": "pool",           # K3 = K2*K tensor-mul
    "a2d": "dve",              # tiny den-coef scale TT
    "uN": ["dve", "dve", "dve", "dve"][: D],  # chain steps (step1 may be act)
    "uD": ["dve", "dve", "dve", "dve"][: D],
    "final": "dve",            # (uN + aN0) * rD
    "x_dma": "sync",
    "out_dma": "sync",
    "scrp_bufs": 8,
}

# SmD column layout: raw accums first (contiguous for the A2d scale TT),
# then the directly-scaled s3. d -> column index in A2d.
if D == 3:
    RAW_DS = [1, 2]          # s1 (tanh accum), s2 (Square accum)
    DCOL = {1: 0, 2: 1, 3: 2}
else:
    RAW_DS = [1, 2, 4]       # s4 free via Square(Q2) accum
    DCOL = {1: 0, 2: 1, 4: 2, 3: 3}


def build_kernel(reps: int = 1, with_bias: bool = True) -> bass.Bass:
    coef = COEFS[D]
    ncden = len(RAW_DS)
    # wcat columns: [W1lo|W1hi|biasQ|cden || W0lo|W0hi|biasK]
    WQ = 2 * H + H + ncden
    WK = 2 * H + H
    WEXT = WQ + WK
    nc = bacc.Bacc("TRN2", target_bir_lowering=False, debug=False)
    xs = nc.declare_dram_parameter("xs", [TC, H], F32, isOutput=False)
    xst = nc.declare_dram_parameter("xst", [128, 2, TC], F32, isOutput=False)
    wcat = nc.declare_dram_parameter("wcat", [128, WEXT], F32, isOutput=False)
    out = nc.declare_dram_parameter("out", [TC, H], F32, isOutput=True)

    with tile.TileContext(nc) as tc, ExitStack() as ctx:
        consts = ctx.enter_context(tc.tile_pool(name="consts", bufs=1))
        io = ctx.enter_context(tc.tile_pool(name="io", bufs=2))
        work = ctx.enter_context(tc.tile_pool(name="work", bufs=2))
        pows = ctx.enter_context(tc.tile_pool(name="pows", bufs=2))
        scrp = ctx.enter_context(tc.tile_pool(name="scrp", bufs=CFG["scrp_bufs"]))
        mom = ctx.enter_context(tc.tile_pool(name="mom", bufs=2))
        psKQ = ctx.enter_context(tc.tile_pool(name="psKQ", bufs=2, space="PSUM"))

        eng = {"dve": nc.vector, "pool": nc.gpsimd, "act": nc.scalar}
        x_eng = getattr(nc, CFG["x_dma"])
        out_eng = getattr(nc, CFG["out_dma"])

        ones1 = consts.tile([1, 128], F32)
        if with_bias:
            nc.gpsimd.memset(ones1, 1.0)
        Xs = []
        XTs = []
        for t0, tl in BLOCKS:
            X = io.tile([128, H], F32, tag=f"X{t0}")
            x_eng.dma_start(out=X[:tl, :], in_=xs[t0 : t0 + tl, :])
            Xs.append(X)
            xT = io.tile([128, 2, 128], F32, tag=f"XT{t0}")
            nc.gpsimd.dma_start(out=xT[:, :, :tl], in_=xst[:, :, t0 : t0 + tl])
            XTs.append(xT)
        wallQ = consts.tile([128, WQ], F32)
        nc.gpsimd.dma_start(out=wallQ, in_=wcat[:, 0:WQ])
        wallK = consts.tile([128, WK], F32)
        nc.gpsimd.dma_start(out=wallK, in_=wcat[:, WQ:WEXT])
        bsbQ = wallQ[0:1, 2 * H : 3 * H]
        bsbK = wallK[0:1, 2 * H : 3 * H]
        cden = wallQ[:, 3 * H : 3 * H + ncden]
        aD0 = float(coef[0]) * float(H)

        def body():
            for bi, (t0, tl) in enumerate(BLOCKS):
                X = Xs[bi]
                xT = XTs[bi]

                # ---- matmuls: queries first (moments only need Q and X)
                psQ = psKQ.tile([128, H], F32, tag="psQ")
                if with_bias:
                    nc.tensor.matmul(
                        psQ[:tl, :], ones1[:, :tl], bsbQ, start=True, stop=False
                    )
                nc.tensor.matmul(
                    psQ[:tl, :], xT[:, 0, :tl], wallQ[:, 0:256],
                    start=not with_bias, stop=False,
                )
                nc.tensor.matmul(
                    psQ[:tl, :], xT[:, 1, :tl], wallQ[:, 256:512],
                    start=False, stop=True,
                )
                psK = psKQ.tile([128, H], F32, tag="psK")
                if with_bias:
                    nc.tensor.matmul(
                        psK[:tl, :], ones1[:, :tl], bsbK, start=True, stop=False
                    )
                nc.tensor.matmul(
                    psK[:tl, :], xT[:, 0, :tl], wallK[:, 0:256],
                    start=not with_bias, stop=False,
                )
                nc.tensor.matmul(
                    psK[:tl, :], xT[:, 1, :tl], wallK[:, 256:512],
                    start=False, stop=True,
                )

                # A2n[:, d] = c_d * m_d (num);  A2d[:, DCOL[d]] = c_d * s_d.
                A2n = mom.tile([128, D + 1], F32, tag="A2n")
                SmD = mom.tile([128, ncden], F32, tag="SmD")
                A2d = mom.tile([128, D], F32, tag="A2d")

                # ---- aN0 = c0 * sum_j x  (independent of everything else)
                j0 = scrp.tile([128, H], F32, tag="scr")
                if CFG["m0"] == "act":
                    nc.scalar.activation(
                        j0[:tl, :], X[:tl, :], AF.Identity,
                        scale=float(coef[0]), accum_out=A2n[:tl, 0:1],
                    )
                else:
                    eng[CFG["m0"]].tensor_scalar(
                        out=j0[:tl, :], in0=X[:tl, :], scalar1=float(coef[0]),
                        scalar2=0.0, op0=OP.mult, op1=OP.add,
                        accum_out=A2n[:tl, 0:1],
                    )

                # ---- ACT: tanh + squares (accum gives raw s1, s2[, s4])
                Qt = work.tile([128, H], F32, tag="Qt")
                nc.scalar.activation(
                    Qt[:tl, :], psQ[:tl, :], AF.Tanh,
                    accum_out=SmD[:tl, 0:1],
                )
                Q = Qt[:tl, :]
                Q2 = pows.tile([128, H], F32, tag="Q2")
                nc.scalar.activation(
                    Q2[:tl, :], Q, AF.Square, accum_out=SmD[:tl, 1:2]
                )
                Kt = work.tile([128, H], F32, tag="Kt")
                nc.scalar.activation(Kt[:tl, :], psK[:tl, :], AF.Tanh)
                K = Kt[:tl, :]
                K2 = pows.tile([128, H], F32, tag="K2")
                nc.scalar.activation(K2[:tl, :], K, AF.Square)
                if D >= 4:
                    Q4 = pows.tile([128, H], F32, tag="Q4")
                    nc.scalar.activation(
                        Q4[:tl, :], Q2[:tl, :], AF.Square,
                        accum_out=SmD[:tl, 2:3],
                    )
                    K4 = pows.tile([128, H], F32, tag="K4")
                    nc.scalar.activation(K4[:tl, :], K2[:tl, :], AF.Square)

                # ---- cubes + remaining den moment
                Q3 = pows.tile([128, H], F32, tag="Q3")
                eng[CFG["qpow3"]].tensor_mul(Q3[:tl, :], Q2[:tl, :], Q)
                K3 = pows.tile([128, H], F32, tag="K3")
                eng[CFG["kpow3"]].tensor_mul(K3[:tl, :], K2[:tl, :], K)
                QP = {1: Q, 2: Q2[:tl, :], 3: Q3[:tl, :]}
                KP = {1: K, 2: K2[:tl, :], 3: K3[:tl, :]}
                if D >= 4:
                    QP[4] = Q4[:tl, :]
                    KP[4] = K4[:tl, :]

                # s3 scaled accum directly into A2d
                j3 = scrp.tile([128, H], F32, tag="scr")
                if CFG["s3"] == "act":
                    nc.scalar.activation(
                        j3[:tl, :], Q3[:tl, :], AF.Identity,
                        scale=float(coef[3]), accum_out=A2d[:tl, DCOL[3] : DCOL[3] + 1],
                    )
                else:
                    nc.vector.tensor_scalar(
                        out=j3[:tl, :], in0=Q3[:tl, :], scalar1=float(coef[3]),
                        scalar2=0.0, op0=OP.mult, op1=OP.add,
                        accum_out=A2d[:tl, DCOL[3] : DCOL[3] + 1],
                    )

                # scale raw den moments by coefficients (tiny TT)
                eng[CFG["a2d"]].tensor_mul(
                    A2d[:tl, 0:ncden], SmD[:tl, :], cden[:tl, :]
                )

                # ---- num moments m_d (coef folded into op scalar)
                for d in range(1, D + 1):
                    mode = CFG["m"][d - 1]
                    if mode == "dve":
                        sd = scrp.tile([128, H], F32, tag="scr")
                        nc.vector.scalar_tensor_tensor(
                            out=sd[:tl, :], in0=QP[d], scalar=float(coef[d]),
                            in1=X[:tl, :], op0=OP.mult, op1=OP.mult,
                            accum_out=A2n[:tl, d : d + 1],
                        )
                    else:
                        vd = scrp.tile([128, H], F32, tag=f"v{d}")
                        nc.gpsimd.tensor_mul(vd[:tl, :], QP[d], X[:tl, :])
                        if mode == "pool_act":
                            jd = scrp.tile([128, H], F32, tag="scr")
                            nc.scalar.activation(
                                jd[:tl, :], vd[:tl, :], AF.Identity,
                                scale=float(coef[d]),
                                accum_out=A2n[:tl, d : d + 1],
                            )
                        else:
                            jd = scrp.tile([128, H], F32, tag="scr")
                            nc.vector.tensor_scalar(
                                out=jd[:tl, :], in0=vd[:tl, :],
                                scalar1=float(coef[d]), scalar2=0.0,
                                op0=OP.mult, op1=OP.add,
                                accum_out=A2n[:tl, d : d + 1],
                            )

                # ---- chains in the power basis (ascending)
                def chain(which_engines, a, tag, bias0=None):
                    # u = a(1)*k [+ bias0]; u += a(d)*k^d for d=2..D
                    u = work.tile([128, H], F32, tag=f"u{tag}")
                    e1 = which_engines[0]
                    if e1 == "act":
                        nc.scalar.activation(
                            u[:tl, :], K, AF.Identity, scale=a(1),
                            bias=0.0 if bias0 is None else bias0,
                        )
                    elif bias0 is None:
                        eng[e1].tensor_scalar(
                            out=u[:tl, :], in0=K, scalar1=a(1),
                            scalar2=None, op0=OP.mult,
                        )
                    else:
                        eng[e1].tensor_scalar(
                            out=u[:tl, :], in0=K, scalar1=a(1),
                            scalar2=bias0, op0=OP.mult, op1=OP.add,
                        )
                    for d in range(2, D + 1):
                        eng[which_engines[d - 1]].scalar_tensor_tensor(
                            out=u[:tl, :], in0=KP[d], scalar=a(d),
                            in1=u[:tl, :], op0=OP.mult, op1=OP.add,
                        )
                    return u

                uD = chain(
                    CFG["uD"], lambda d: A2d[:tl, DCOL[d] : DCOL[d] + 1],
                    "d", bias0=aD0,
                )
                uN = chain(CFG["uN"], lambda d: A2n[:tl, d : d + 1], "n")

                # ---- out = (uN + aN0) * (1/den)
                rD = work.tile([128, H], F32, tag="rD")
                nc.vector.reciprocal_approx_fast(rD[:tl, :], uD[:tl, :])
                O = io.tile([128, H], F32, tag="O")
                eng[CFG["final"]].scalar_tensor_tensor(
                    out=O[:tl, :], in0=uN[:tl, :], scalar=A2n[:tl, 0:1],
                    in1=rD[:tl, :], op0=OP.add, op1=OP.mult,
                )
                out_eng.dma_start(out=out[t0 : t0 + tl, :], in_=O[:tl, :])

        if reps == 1:
            body()
        else:
            with tc.For_i(0, reps, 1):
                body()

    nc.compile()
    return nc


_NCS = {}


def _get_nc(with_bias: bool = True):
    if with_bias not in _NCS:
        _NCS[with_bias] = build_kernel(with_bias=with_bias)
    return _NCS[with_bias]


def _make_in_maps(x, W0, b0, W1, b1):
    ncden = len(RAW_DS)
    coef = COEFS[D]
    xf = np.ascontiguousarray(np.asarray(x, np.float32).reshape(T, H))
    W0 = np.asarray(W0, np.float32)
    W1 = np.asarray(W1, np.float32)
    biasQ = np.zeros((128, H), np.float32)
    biasQ[0, :] = np.asarray(b1, np.float32)
    biasK = np.zeros((128, H), np.float32)
    biasK[0, :] = np.asarray(b0, np.float32)
    cden = np.tile(
        np.array([coef[d] for d in RAW_DS], np.float32).reshape(1, ncden),
        (128, 1),
    )
    wcat = np.ascontiguousarray(
        np.concatenate(
            [W1[:128, :], W1[128:, :], biasQ, cden,
             W0[:128, :], W0[128:, :], biasK],
            axis=1,
        )
    )  # [128, WQ+WK]
    maps = []
    for c in range(NCORES):
        sh = np.ascontiguousarray(xf[c * TC : (c + 1) * TC])  # [TC, H]
        # xst[h, chunk, t] = sh[t, chunk*128 + h]
        xst = np.ascontiguousarray(
            np.transpose(sh.reshape(TC, 2, 128), (2, 1, 0))
        )
        maps.append({"xs": sh, "xst": xst, "wcat": wcat})
    return maps


def _ensure_axon():
    try:
        import jax
        if not any(d.platform == "axon" for d in jax.devices()):
            jax.config.update("jax_platforms", "axon,cpu")
    except Exception:
        pass


def _run(x, W0, b0, W1, b1, trace=False, **kw):
    _ensure_axon()
    with_bias = bool(
        np.any(np.asarray(b0, np.float32)) or np.any(np.asarray(b1, np.float32))
    )
    res = run_bass_kernel_spmd(
        _get_nc(with_bias), _make_in_maps(x, W0, b0, W1, b1),
        list(range(NCORES)), trace=trace, **kw,
    )
    outs = [res.results[c]["out"] for c in range(NCORES)]
    full = np.concatenate(outs, axis=0).reshape(B, S, M, H).astype(np.float32)
    return full, res


def kernel(x, W0, b0, W1, b1):
    full, _ = _run(x, W0, b0, W1, b1, trace=False)
    return full


# revision 16
# speedup vs baseline: 1.5984x; 1.4553x over previous
"""Trainium2 Bass kernel for per-token outer-product softmax attention.

Reference computation (per token t of 1600, H=256):
    k = tanh(x W0 + b0);  q = tanh(x W1 + b1)
    scores[i,j] = k[i]*q[j];  attn = softmax_j(scores);  out = attn @ x

Key algebra: k,q are tanh outputs so k[i]*q[j] in (-1,1). On [-1,1],
exp(s) is approximated well below the 2e-2 output tolerance by a
low-degree minimax polynomial P(s) = sum_d c_d s^d, and P(k_i q_j) =
sum_d c_d k_i^d q_j^d is SEPARABLE. Softmax numerator/denominator become
per-token moments:
    num_i = sum_d (c_d sum_j q_j^d x_j) k_i^d
    den_i = sum_d (c_d sum_j q_j^d)     k_i^d
so the 256x256 scores tensor is never materialized. Moments come free as
accum_out of the product ops (coefficients folded into the op scalars);
the k-polynomials are evaluated in the power basis (powers via ACT
Square / Pool muls) as short multiply-accumulate sequences spread across
DVE/ACT/Pool. Working dtype is fp16 (output fp32): end-to-end rel-L2 err
~3e-3 at D=3.

Sharding: pure data parallel over tokens, 200 tokens/core x 8 cores;
weights replicated.
"""

import numpy as np
from contextlib import ExitStack

import concourse.bass as bass
import concourse.bacc as bacc
import concourse.tile as tile
from concourse import mybir
from concourse.bass_utils import run_bass_kernel_spmd

F32 = mybir.dt.float32
F16 = mybir.dt.float16
AF = mybir.ActivationFunctionType
OP = mybir.AluOpType

B, S, M, H = 4, 10, 40, 256
T = B * S * M            # 1600 tokens
NCORES = 8
TC = T // NCORES         # 200 tokens per core
BLOCKS = [(0, 128), (128, TC - 128)]

# Minimax-relative-error coefficients (monomial basis) of exp on [-1,1].
# Poly max rel err: D=3 -> 5.0e-3, D=4 -> 5.0e-4; end-to-end output
# rel-L2 err: D=3 fp16 ~3e-3, D=4 fp32 2.9e-4 (tolerance 2e-2).
COEFS = {
    3: [0.99650635, 1.0107962638, 0.5388581246, 0.1585305384],
    4: [0.9996280079, 0.9979377479, 0.5028966853, 0.1764876527,
        0.0399652955],
}

D = 3

# Placement/config knobs (tuned via CoreSim + HW loop benchmarks).
CFG = {
    "dtype": "f16",           # working dtype for powers/chains/products
    "q2": "dve",              # act (Square, free s2 accum) | pool | dve
    "k2": "pool",             # act | pool | dve
    "q3": "pool",             # pool | dve
    "k3": "pool",
    "s2": "dve",              # free (q2==act) | dve (TS-acc) | act (Id-acc)
    "s3": "dve",              # dve | act
    "m0": "act",              # dve | act
    # m1..mD: "stt" (DVE fused) | "pool_dve" (Pool product + DVE TS-acc)
    #         | "pool_act" (Pool product + ACT Id-acc)
    "m": ["pool_dve", "pool_act", "pool_dve", "stt"][: D],
    # chain steps 1..D: step1: "dve" (TS) | "act" (Identity scale+bias);
    # steps>=2: "stt" (DVE fused) | "ts_pool" (DVE TS + Pool add)
    #           | "ts_dve" (DVE TS + DVE add)
    "uN": ["dve", "ts_pool", "ts_pool", "stt"][: D],
    "uD": ["dve", "stt", "ts_pool", "stt"][: D],
    "final": "stt",           # stt (DVE) | ts_pool (DVE TS + Pool mult)
    "interleave": True,       # emit b0 head, b1 head, b0 tail, b1 tail
    "w_dma": "gpsimd",        # queue for weight DMAs: sync | scalar | gpsimd
    "x_dma": "sync",
    "out_dma": "sync",
    "scrp_bufs": 8,
}


def build_kernel(reps: int = 1, with_bias: bool = True) -> bass.Bass:
    coef = COEFS[D]
    FW = F16 if CFG["dtype"] == "f16" else F32
    # raw den-moment columns needing the cden scale TT
    raw_ds = [1] + ([2] if CFG["q2"] == "act" and CFG["s2"] == "free" else [])
    ncden = len(raw_ds)
    # col layout in A2d: raw cols first, then direct-scaled cols
    dcol = {}
    for i, d in enumerate(raw_ds):
        dcol[d] = i
    nxt = ncden
    for d in range(2, D + 1):
        if d not in dcol:
            dcol[d] = nxt
            nxt += 1

    WW = 2 * H + H + 2 * H + H   # [W1lo|W1hi|biasQ || W0lo|W0hi|biasK] fp16
    nc = bacc.Bacc("TRN2", target_bir_lowering=False, debug=False)
    xs = nc.declare_dram_parameter("xs", [TC, H], FW, isOutput=False)
    xst = nc.declare_dram_parameter("xst", [128, 2, TC], FW, isOutput=False)
    wcat = nc.declare_dram_parameter("wcat", [128, WW], FW, isOutput=False)
    cdn = nc.declare_dram_parameter("cdn", [128, max(ncden, 1)], F32, isOutput=False)
    out = nc.declare_dram_parameter("out", [TC, H], F32, isOutput=True)

    with tile.TileContext(nc) as tc, ExitStack() as ctx:
        consts = ctx.enter_context(tc.tile_pool(name="consts", bufs=1))
        io = ctx.enter_context(tc.tile_pool(name="io", bufs=2))
        work = ctx.enter_context(tc.tile_pool(name="work", bufs=2))
        pows = ctx.enter_context(tc.tile_pool(name="pows", bufs=2))
        scrp = ctx.enter_context(tc.tile_pool(name="scrp", bufs=CFG["scrp_bufs"]))
        mom = ctx.enter_context(tc.tile_pool(name="mom", bufs=2))
        psKQ = ctx.enter_context(tc.tile_pool(name="psKQ", bufs=2, space="PSUM"))

        eng = {"dve": nc.vector, "pool": nc.gpsimd, "act": nc.scalar}
        x_eng = getattr(nc, CFG["x_dma"])
        out_eng = getattr(nc, CFG["out_dma"])

        ones1 = consts.tile([1, 128], FW)
        if with_bias:
            nc.gpsimd.memset(ones1, 1.0)
        Xs = []
        XTs = []
        for t0, tl in BLOCKS:
            X = io.tile([128, H], FW, tag=f"X{t0}")
            x_eng.dma_start(out=X[:tl, :], in_=xs[t0 : t0 + tl, :])
            Xs.append(X)
            xT = io.tile([128, 2, 128], FW, tag=f"XT{t0}")
            nc.gpsimd.dma_start(out=xT[:, :, :tl], in_=xst[:, :, t0 : t0 + tl])
            XTs.append(xT)
        w_eng = getattr(nc, CFG["w_dma"])
        wallQ = consts.tile([128, 3 * H], FW)
        w_eng.dma_start(out=wallQ, in_=wcat[:, 0 : 3 * H])
        wallK = consts.tile([128, 3 * H], FW)
        w_eng.dma_start(out=wallK, in_=wcat[:, 3 * H : 6 * H])
        cden = consts.tile([128, max(ncden, 1)], F32)
        w_eng.dma_start(out=cden, in_=cdn[:, :])
        bsbQ = wallQ[0:1, 2 * H : 3 * H]
        bsbK = wallK[0:1, 2 * H : 3 * H]
        aD0 = float(coef[0]) * float(H)

        def head(bi):
            t0, tl = BLOCKS[bi]
            if True:
                X = Xs[bi]
                xT = XTs[bi]

                # ---- matmuls: queries first (moments only need Q and X)
                psQ = psKQ.tile([128, H], F32, tag="psQ")
                if with_bias:
                    nc.tensor.matmul(
                        psQ[:tl, :], ones1[:, :tl], bsbQ, start=True, stop=False
                    )
                nc.tensor.matmul(
                    psQ[:tl, :], xT[:, 0, :tl], wallQ[:, 0:256],
                    start=not with_bias, stop=False,
                )
                nc.tensor.matmul(
                    psQ[:tl, :], xT[:, 1, :tl], wallQ[:, 256:512],
                    start=False, stop=True,
                )
                psK = psKQ.tile([128, H], F32, tag="psK")
                if with_bias:
                    nc.tensor.matmul(
                        psK[:tl, :], ones1[:, :tl], bsbK, start=True, stop=False
                    )
                nc.tensor.matmul(
                    psK[:tl, :], xT[:, 0, :tl], wallK[:, 0:256],
                    start=not with_bias, stop=False,
                )
                nc.tensor.matmul(
                    psK[:tl, :], xT[:, 1, :tl], wallK[:, 256:512],
                    start=False, stop=True,
                )

                # A2n[:, d] = c_d * m_d ; A2d[:, dcol[d]] = c_d * s_d
                A2n = mom.tile([128, D + 1], F32, tag="A2n")
                SmD = mom.tile([128, max(ncden, 1)], F32, tag="SmD")
                A2d = mom.tile([128, D], F32, tag="A2d")

                # ---- aN0 = c0 * sum_j x
                j0 = scrp.tile([128, H], FW, tag="scr")
                if CFG["m0"] == "act":
                    nc.scalar.activation(
                        j0[:tl, :], X[:tl, :], AF.Identity,
                        scale=float(coef[0]), accum_out=A2n[:tl, 0:1],
                    )
                else:
                    nc.vector.tensor_scalar(
                        out=j0[:tl, :], in0=X[:tl, :], scalar1=float(coef[0]),
                        scalar2=0.0, op0=OP.mult, op1=OP.add,
                        accum_out=A2n[:tl, 0:1],
                    )

                # ---- tanh (accum gives raw s1)
                Qt = work.tile([128, H], FW, tag="Qt")
                nc.scalar.activation(
                    Qt[:tl, :], psQ[:tl, :], AF.Tanh, accum_out=SmD[:tl, 0:1]
                )
                Q = Qt[:tl, :]
                Kt = work.tile([128, H], FW, tag="Kt")
                nc.scalar.activation(Kt[:tl, :], psK[:tl, :], AF.Tanh)
                K = Kt[:tl, :]

                # ---- powers
                Q2 = pows.tile([128, H], FW, tag="Q2")
                if CFG["q2"] == "act":
                    kw = (
                        {"accum_out": SmD[:tl, dcol[2] : dcol[2] + 1]}
                        if CFG["s2"] == "free" else {}
                    )
                    nc.scalar.activation(Q2[:tl, :], Q, AF.Square, **kw)
                else:
                    eng[CFG["q2"]].tensor_mul(Q2[:tl, :], Q, Q)
                K2 = pows.tile([128, H], FW, tag="K2")
                if CFG["k2"] == "act":
                    nc.scalar.activation(K2[:tl, :], K, AF.Square)
                else:
                    eng[CFG["k2"]].tensor_mul(K2[:tl, :], K, K)
                Q3 = pows.tile([128, H], FW, tag="Q3")
                eng[CFG["q3"]].tensor_mul(Q3[:tl, :], Q2[:tl, :], Q)
                K3 = pows.tile([128, H], FW, tag="K3")
                eng[CFG["k3"]].tensor_mul(K3[:tl, :], K2[:tl, :], K)
                QP = {1: Q, 2: Q2[:tl, :], 3: Q3[:tl, :]}
                KP = {1: K, 2: K2[:tl, :], 3: K3[:tl, :]}
                if D >= 4:
                    Q4 = pows.tile([128, H], FW, tag="Q4")
                    nc.scalar.activation(Q4[:tl, :], Q2[:tl, :], AF.Square)
                    K4 = pows.tile([128, H], FW, tag="K4")
                    nc.scalar.activation(K4[:tl, :], K2[:tl, :], AF.Square)
                    QP[4] = Q4[:tl, :]
                    KP[4] = K4[:tl, :]

                # ---- scaled den moments s_d -> A2d (direct for non-raw)
                def den_accum(d):
                    js = scrp.tile([128, H], FW, tag="scr")
                    tgt = A2d[:tl, dcol[d] : dcol[d] + 1]
                    mode = CFG["s2"] if d == 2 else CFG["s3"]
                    if mode == "act":
                        nc.scalar.activation(
                            js[:tl, :], QP[d], AF.Identity,
                            scale=float(coef[d]), accum_out=tgt,
                        )
                    else:
                        nc.vector.tensor_scalar(
                            out=js[:tl, :], in0=QP[d], scalar1=float(coef[d]),
                            scalar2=0.0, op0=OP.mult, op1=OP.add,
                            accum_out=tgt,
                        )

                for d in range(2, D + 1):
                    if d in raw_ds:
                        continue
                    den_accum(d)

                # scale raw den moments by coefficients (tiny TT)
                nc.vector.tensor_mul(
                    A2d[:tl, 0:ncden], SmD[:tl, 0:ncden], cden[:tl, 0:ncden]
                )

                # ---- num moments m_d (coef folded into op scalar)
                for d in range(1, D + 1):
                    mode = CFG["m"][d - 1]
                    if mode == "stt":
                        sd = scrp.tile([128, H], FW, tag="scr")
                        nc.vector.scalar_tensor_tensor(
                            out=sd[:tl, :], in0=QP[d], scalar=float(coef[d]),
                            in1=X[:tl, :], op0=OP.mult, op1=OP.mult,
                            accum_out=A2n[:tl, d : d + 1],
                        )
                    else:
                        vd = scrp.tile([128, H], FW, tag=f"v{d}")
                        nc.gpsimd.tensor_mul(vd[:tl, :], QP[d], X[:tl, :])
                        jd = scrp.tile([128, H], FW, tag="scr")
                        if mode == "pool_act":
                            nc.scalar.activation(
                                jd[:tl, :], vd[:tl, :], AF.Identity,
                                scale=float(coef[d]),
                                accum_out=A2n[:tl, d : d + 1],
                            )
                        else:
                            nc.vector.tensor_scalar(
                                out=jd[:tl, :], in0=vd[:tl, :],
                                scalar1=float(coef[d]), scalar2=0.0,
                                op0=OP.mult, op1=OP.add,
                                accum_out=A2n[:tl, d : d + 1],
                            )

                return {"X": X, "KP": KP, "A2n": A2n, "A2d": A2d, "dcol": dcol}

        def tail(bi, st):
            t0, tl = BLOCKS[bi]
            if True:
                X, KP, A2n, A2d = st["X"], st["KP"], st["A2n"], st["A2d"]
                K = KP[1]

                # ---- chains in the power basis (ascending)
                def chain(modes, a, tag, bias0=None, last_f32=False):
                    u = work.tile([128, H], FW, tag=f"u{tag}")
                    if modes[0] == "act":
                        nc.scalar.activation(
                            u[:tl, :], K, AF.Identity, scale=a(1),
                            bias=0.0 if bias0 is None else bias0,
                        )
                    elif bias0 is None:
                        nc.vector.tensor_scalar(
                            out=u[:tl, :], in0=K, scalar1=a(1),
                            scalar2=None, op0=OP.mult,
                        )
                    else:
                        nc.vector.tensor_scalar(
                            out=u[:tl, :], in0=K, scalar1=a(1),
                            scalar2=bias0, op0=OP.mult, op1=OP.add,
                        )
                    cur = u
                    for d in range(2, D + 1):
                        last = d == D
                        odt = F32 if (last and last_f32) else FW
                        mode = modes[d - 1]
                        nxt_t = work.tile([128, H], odt, tag=f"u{tag}{d}")
                        if mode == "stt":
                            nc.vector.scalar_tensor_tensor(
                                out=nxt_t[:tl, :], in0=KP[d], scalar=a(d),
                                in1=cur[:tl, :], op0=OP.mult, op1=OP.add,
                            )
                        else:
                            td = scrp.tile([128, H], FW, tag="scr")
                            nc.vector.tensor_scalar(
                                out=td[:tl, :], in0=KP[d], scalar1=a(d),
                                scalar2=None, op0=OP.mult,
                            )
                            add_eng = nc.gpsimd if mode == "ts_pool" else nc.vector
                            add_eng.tensor_add(
                                nxt_t[:tl, :], td[:tl, :], cur[:tl, :]
                            )
                        cur = nxt_t
                    return cur

                uD = chain(
                    CFG["uD"], lambda d: A2d[:tl, dcol[d] : dcol[d] + 1],
                    "d", bias0=aD0, last_f32=True,
                )
                uN = chain(CFG["uN"], lambda d: A2n[:tl, d : d + 1], "n")

                # ---- out = (uN + aN0) * (1/den)
                rD = work.tile([128, H], F32, tag="rD")
                nc.vector.reciprocal_approx_fast(rD[:tl, :], uD[:tl, :])
                O = io.tile([128, H], F32, tag="O")
                if CFG["final"] == "stt":
                    nc.vector.scalar_tensor_tensor(
                        out=O[:tl, :], in0=uN[:tl, :], scalar=A2n[:tl, 0:1],
                        in1=rD[:tl, :], op0=OP.add, op1=OP.mult,
                    )
                else:
                    tf = scrp.tile([128, H], FW, tag="scr")
                    nc.vector.tensor_scalar(
                        out=tf[:tl, :], in0=uN[:tl, :], scalar1=A2n[:tl, 0:1],
                        scalar2=None, op0=OP.add,
                    )
                    nc.gpsimd.tensor_mul(O[:tl, :], tf[:tl, :], rD[:tl, :])
                out_eng.dma_start(out=out[t0 : t0 + tl, :], in_=O[:tl, :])

        def body():
            if CFG["interleave"]:
                s0 = head(0)
                s1 = head(1)
                tail(0, s0)
                tail(1, s1)
            else:
                for bi in range(len(BLOCKS)):
                    tail(bi, head(bi))

        if reps == 1:
            body()
        else:
            with tc.For_i(0, reps, 1):
                body()

    nc.compile()
    return nc


_NCS = {}


def _get_nc(with_bias: bool = True):
    if with_bias not in _NCS:
        _NCS[with_bias] = build_kernel(with_bias=with_bias)
    return _NCS[with_bias]


def _make_in_maps(x, W0, b0, W1, b1):
    coef = COEFS[D]
    raw_ds = [1] + ([2] if CFG["q2"] == "act" and CFG["s2"] == "free" else [])
    ncden = len(raw_ds)
    npw = np.float16 if CFG["dtype"] == "f16" else np.float32
    xf = np.ascontiguousarray(np.asarray(x, np.float32).reshape(T, H))
    W0 = np.asarray(W0, np.float32).astype(npw)
    W1 = np.asarray(W1, np.float32).astype(npw)
    biasQ = np.zeros((128, H), npw)
    biasQ[0, :] = np.asarray(b1, np.float32).astype(npw)
    biasK = np.zeros((128, H), npw)
    biasK[0, :] = np.asarray(b0, np.float32).astype(npw)
    cdn = np.tile(
        np.array([coef[d] for d in raw_ds], np.float32).reshape(1, ncden),
        (128, 1),
    ).astype(np.float32)
    wcat = np.ascontiguousarray(
        np.concatenate(
            [W1[:128, :], W1[128:, :], biasQ, W0[:128, :], W0[128:, :], biasK],
            axis=1,
        )
    )  # [128, 6H] fp16
    maps = []
    for c in range(NCORES):
        sh = np.ascontiguousarray(xf[c * TC : (c + 1) * TC]).astype(npw)
        xst = np.ascontiguousarray(
            np.transpose(sh.reshape(TC, 2, 128), (2, 1, 0))
        )
        maps.append({"xs": sh, "xst": xst, "wcat": wcat, "cdn": cdn})
    return maps


def _ensure_axon():
    try:
        import jax
        if not any(d.platform == "axon" for d in jax.devices()):
            jax.config.update("jax_platforms", "axon,cpu")
    except Exception:
        pass


def _run(x, W0, b0, W1, b1, trace=False, **kw):
    _ensure_axon()
    with_bias = bool(
        np.any(np.asarray(b0, np.float32)) or np.any(np.asarray(b1, np.float32))
    )
    res = run_bass_kernel_spmd(
        _get_nc(with_bias), _make_in_maps(x, W0, b0, W1, b1),
        list(range(NCORES)), trace=trace, **kw,
    )
    outs = [res.results[c]["out"] for c in range(NCORES)]
    full = np.concatenate(outs, axis=0).reshape(B, S, M, H).astype(np.float32)
    return full, res


def kernel(x, W0, b0, W1, b1):
    full, _ = _run(x, W0, b0, W1, b1, trace=False)
    return full


# revision 32
# speedup vs baseline: 1.7497x; 1.0946x over previous
"""Trainium2 Bass kernel for per-token outer-product softmax attention.

Reference computation (per token t of 1600, H=256):
    k = tanh(x W0 + b0);  q = tanh(x W1 + b1)
    scores[i,j] = k[i]*q[j];  attn = softmax_j(scores);  out = attn @ x

Key algebra: k,q are tanh outputs so k[i]*q[j] in (-1,1). On [-1,1],
exp(s) is approximated well below the 2e-2 output tolerance by a
low-degree minimax polynomial P(s) = sum_d c_d s^d, and P(k_i q_j) =
sum_d c_d k_i^d q_j^d is SEPARABLE. Softmax numerator/denominator become
per-token moments:
    num_i = sum_d (c_d sum_j q_j^d x_j) k_i^d
    den_i = sum_d (c_d sum_j q_j^d)     k_i^d
so the 256x256 scores tensor is never materialized. Moments come free as
accum_out of the product ops (coefficients folded into the op scalars);
the k-polynomials are evaluated in the power basis (powers via ACT
Square / Pool muls) as short multiply-accumulate sequences spread across
DVE/ACT/Pool. Working dtype is fp16 (output fp32): end-to-end rel-L2 err
~3e-3 at D=3.

Sharding: pure data parallel over tokens, 200 tokens/core x 8 cores;
weights replicated.
"""

import numpy as np
from contextlib import ExitStack

import concourse.bass as bass
import concourse.bacc as bacc
import concourse.tile as tile
from concourse import mybir
from concourse.bass_utils import run_bass_kernel_spmd

F32 = mybir.dt.float32
F16 = mybir.dt.float16
AF = mybir.ActivationFunctionType
OP = mybir.AluOpType

B, S, M, H = 4, 10, 40, 256
T = B * S * M            # 1600 tokens
NCORES = 8
TC = T // NCORES         # 200 tokens per core
BLOCKS = [(0, 128), (128, TC - 128)]

# Minimax-relative-error coefficients (monomial basis) of exp on [-1,1].
# Poly max rel err: D=3 -> 5.0e-3, D=4 -> 5.0e-4; end-to-end output
# rel-L2 err: D=3 fp16 ~3e-3, D=4 fp32 2.9e-4 (tolerance 2e-2).
COEFS = {
    3: [0.99650635, 1.0107962638, 0.5388581246, 0.1585305384],
    4: [0.9996280079, 0.9979377479, 0.5028966853, 0.1764876527,
        0.0399652955],
}

D = 3

# Placement/config knobs (tuned via CoreSim + HW loop benchmarks).
CFG = {
    "dtype": "f16",           # working dtype for powers/chains/products
    "q2": "act",              # act (Square, free s2 accum) | pool | dve
    "k2": "pool",             # act | pool | dve
    "q3": "pool",             # pool | dve
    "k3": "pool",
    "s1": "dve",              # free (tanh accum + scale TT) | dve (TS-acc)
    "s2": "dve",              # free (q2==act) | dve (TS-acc) | act (Id-acc)
    "s3": "dve",              # dve | act
    "m0": "pe",               # pe (ones-matmul) | dve | act
    # m1..mD: "ttr" (DVE tensor_tensor_reduce, 1 op) | "stt" (DVE fused)
    #         | "pool_dve" (Pool product + DVE TS-acc)
    #         | "pool_act" (Pool product + ACT Id-acc)
    # NOTE: "ttr" (tensor_tensor_reduce) crashes on HW — do not use.
    "m": ["stt", "stt", "stt", "stt"][: D],
    # chain form: "power" (ascending power basis, needs K2/K3) or
    # "horner" (descending, all-DVE TS+STT, no K powers needed)
    "uN_form": "horner",
    "uD_form": "power",
    # power-form steps 1..D: step1: "dve" (TS) | "act" (Id scale+bias);
    # steps>=2: "stt" (DVE fused) | "ts_pool" (DVE TS + Pool add)
    #           | "ts_dve" (DVE TS + DVE add)
    "uN": ["dve", "stt", "stt", "stt"][: D],
    "uD": ["dve", "stt", "stt", "stt"][: D],
    "final": "stt",           # stt (DVE) | ts_pool (DVE TS + Pool mult)
    "interleave": True,       # emit b0 head, b1 head, b0 tail, b1 tail
    "w_dma": "gpsimd",        # queue for weight DMAs: sync | scalar | gpsimd
    "x_dma": "sync",
    "out_dma": "sync",
    "io_bufs": 4,
    "work_bufs": 4,
    "pows_bufs": 4,
    "mom_bufs": 4,
    "ps_bufs": 3,
    "scrp_bufs": 16,
}


def build_kernel(reps: int = 1, with_bias: bool = True) -> bass.Bass:
    coef = COEFS[D]
    FW = F16 if CFG["dtype"] == "f16" else F32
    # raw den-moment columns needing the cden scale TT
    raw_ds = [d for d, k in [(1, "s1"), (2, "s2")] if CFG[k] == "free"]
    ncden = len(raw_ds)
    # col layout in A2d: raw cols first, then direct-scaled cols
    dcol = {}
    for i, d in enumerate(raw_ds):
        dcol[d] = i
    nxt = ncden
    for d in range(1, D + 1):
        if d not in dcol:
            dcol[d] = nxt
            nxt += 1

    WW = 2 * H + H + 2 * H + H   # [W1lo|W1hi|biasQ || W0lo|W0hi|biasK] fp16
    nc = bacc.Bacc("TRN2", target_bir_lowering=False, debug=False)
    xs = nc.declare_dram_parameter("xs", [TC, H], FW, isOutput=False)
    xst = nc.declare_dram_parameter("xst", [128, 2, TC], FW, isOutput=False)
    wcat = nc.declare_dram_parameter("wcat", [128, WW], FW, isOutput=False)
    cdn = nc.declare_dram_parameter("cdn", [128, max(ncden, 1)], F32, isOutput=False)
    out = nc.declare_dram_parameter("out", [TC, H], F32, isOutput=True)

    with tile.TileContext(nc) as tc, ExitStack() as ctx:
        consts = ctx.enter_context(tc.tile_pool(name="consts", bufs=1))
        io = ctx.enter_context(tc.tile_pool(name="io", bufs=CFG["io_bufs"]))
        work = ctx.enter_context(tc.tile_pool(name="work", bufs=CFG["work_bufs"]))
        pows = ctx.enter_context(tc.tile_pool(name="pows", bufs=CFG["pows_bufs"]))
        scrp = ctx.enter_context(tc.tile_pool(name="scrp", bufs=CFG["scrp_bufs"]))
        mom = ctx.enter_context(tc.tile_pool(name="mom", bufs=CFG["mom_bufs"]))
        psKQ = ctx.enter_context(
            tc.tile_pool(name="psKQ", bufs=CFG["ps_bufs"], space="PSUM")
        )

        eng = {"dve": nc.vector, "pool": nc.gpsimd, "act": nc.scalar}
        x_eng = getattr(nc, CFG["x_dma"])
        out_eng = getattr(nc, CFG["out_dma"])

        ones1 = consts.tile([1, 128], FW)
        if with_bias:
            nc.gpsimd.memset(ones1, 1.0)
        if CFG["m0"] == "pe":
            c0col = consts.tile([128, 1], FW)
            nc.gpsimd.memset(c0col, float(coef[0]))
            psM = ctx.enter_context(tc.tile_pool(name="psM", bufs=2, space="PSUM"))
        Xs = []
        XTs = []
        for t0, tl in BLOCKS:
            X = io.tile([128, H], FW, tag=f"X{t0}")
            x_eng.dma_start(out=X[:tl, :], in_=xs[t0 : t0 + tl, :])
            Xs.append(X)
            xT = io.tile([128, 2, 128], FW, tag=f"XT{t0}")
            nc.gpsimd.dma_start(out=xT[:, :, :tl], in_=xst[:, :, t0 : t0 + tl])
            XTs.append(xT)
        w_eng = getattr(nc, CFG["w_dma"])
        wallQ = consts.tile([128, 3 * H], FW)
        w_eng.dma_start(out=wallQ, in_=wcat[:, 0 : 3 * H])
        wallK = consts.tile([128, 3 * H], FW)
        w_eng.dma_start(out=wallK, in_=wcat[:, 3 * H : 6 * H])
        cden = consts.tile([128, max(ncden, 1)], F32)
        w_eng.dma_start(out=cden, in_=cdn[:, :])
        bsbQ = wallQ[0:1, 2 * H : 3 * H]
        bsbK = wallK[0:1, 2 * H : 3 * H]
        aD0 = float(coef[0]) * float(H)

        def head(bi):
            t0, tl = BLOCKS[bi]
            if True:
                X = Xs[bi]
                xT = XTs[bi]

                # ---- matmuls: queries first (moments only need Q and X)
                psQ = psKQ.tile([128, H], F32, tag="psQ")
                if with_bias:
                    nc.tensor.matmul(
                        psQ[:tl, :], ones1[:, :tl], bsbQ, start=True, stop=False
                    )
                nc.tensor.matmul(
                    psQ[:tl, :], xT[:, 0, :tl], wallQ[:, 0:256],
                    start=not with_bias, stop=False,
                )
                nc.tensor.matmul(
                    psQ[:tl, :], xT[:, 1, :tl], wallQ[:, 256:512],
                    start=False, stop=True,
                )
                psK = psKQ.tile([128, H], F32, tag="psK")
                if with_bias:
                    nc.tensor.matmul(
                        psK[:tl, :], ones1[:, :tl], bsbK, start=True, stop=False
                    )
                nc.tensor.matmul(
                    psK[:tl, :], xT[:, 0, :tl], wallK[:, 0:256],
                    start=not with_bias, stop=False,
                )
                nc.tensor.matmul(
                    psK[:tl, :], xT[:, 1, :tl], wallK[:, 256:512],
                    start=False, stop=True,
                )

                # A2n[:, d] = c_d * m_d ; A2d[:, dcol[d]] = c_d * s_d
                A2n = mom.tile([128, D + 1], F32, tag="A2n")
                SmD = mom.tile([128, max(ncden, 1)], F32, tag="SmD")
                A2d = mom.tile([128, D], F32, tag="A2d")

                # ---- aN0 = c0 * sum_j x
                if CFG["m0"] == "pe":
                    psM0 = psM.tile([128, 1], F32, tag="psM0")
                    nc.tensor.matmul(
                        psM0[:tl, :], xT[:, 0, :tl], c0col,
                        start=True, stop=False,
                    )
                    nc.tensor.matmul(
                        psM0[:tl, :], xT[:, 1, :tl], c0col,
                        start=False, stop=True,
                    )
                    aN0 = psM0[:tl, 0:1]
                else:
                    j0 = scrp.tile([128, H], FW, tag="scr")
                    if CFG["m0"] == "act":
                        nc.scalar.activation(
                            j0[:tl, :], X[:tl, :], AF.Identity,
                            scale=float(coef[0]), accum_out=A2n[:tl, 0:1],
                        )
                    else:
                        nc.vector.tensor_scalar(
                            out=j0[:tl, :], in0=X[:tl, :], scalar1=float(coef[0]),
                            scalar2=0.0, op0=OP.mult, op1=OP.add,
                            accum_out=A2n[:tl, 0:1],
                        )
                    aN0 = A2n[:tl, 0:1]

                # ---- tanh (accum gives raw s1 when s1 == "free")
                Qt = work.tile([128, H], FW, tag="Qt")
                kw1 = (
                    {"accum_out": SmD[:tl, dcol[1] : dcol[1] + 1]}
                    if CFG["s1"] == "free" else {}
                )
                nc.scalar.activation(Qt[:tl, :], psQ[:tl, :], AF.Tanh, **kw1)
                Q = Qt[:tl, :]
                Kt = work.tile([128, H], FW, tag="Kt")
                nc.scalar.activation(Kt[:tl, :], psK[:tl, :], AF.Tanh)
                K = Kt[:tl, :]

                # ---- powers
                Q2 = pows.tile([128, H], FW, tag="Q2")
                if CFG["q2"] == "act":
                    kw = (
                        {"accum_out": SmD[:tl, dcol[2] : dcol[2] + 1]}
                        if CFG["s2"] == "free" else {}
                    )
                    nc.scalar.activation(Q2[:tl, :], Q, AF.Square, **kw)
                else:
                    eng[CFG["q2"]].tensor_mul(Q2[:tl, :], Q, Q)
                need_kp = "power" in (CFG["uN_form"], CFG["uD_form"])
                KP = {1: K}
                if need_kp:
                    K2 = pows.tile([128, H], FW, tag="K2")
                    if CFG["k2"] == "act":
                        nc.scalar.activation(K2[:tl, :], K, AF.Square)
                    else:
                        eng[CFG["k2"]].tensor_mul(K2[:tl, :], K, K)
                    KP[2] = K2[:tl, :]
                Q3 = pows.tile([128, H], FW, tag="Q3")
                eng[CFG["q3"]].tensor_mul(Q3[:tl, :], Q2[:tl, :], Q)
                if need_kp:
                    K3 = pows.tile([128, H], FW, tag="K3")
                    eng[CFG["k3"]].tensor_mul(K3[:tl, :], K2[:tl, :], K)
                    KP[3] = K3[:tl, :]
                QP = {1: Q, 2: Q2[:tl, :], 3: Q3[:tl, :]}
                if D >= 4:
                    Q4 = pows.tile([128, H], FW, tag="Q4")
                    nc.scalar.activation(Q4[:tl, :], Q2[:tl, :], AF.Square)
                    K4 = pows.tile([128, H], FW, tag="K4")
                    nc.scalar.activation(K4[:tl, :], K2[:tl, :], AF.Square)
                    QP[4] = Q4[:tl, :]
                    KP[4] = K4[:tl, :]

                # ---- scaled den moments s_d -> A2d (direct for non-raw)
                def den_accum(d):
                    js = scrp.tile([128, H], FW, tag="scr")
                    tgt = A2d[:tl, dcol[d] : dcol[d] + 1]
                    mode = CFG["s" + str(min(d, 3))]
                    if mode == "act":
                        nc.scalar.activation(
                            js[:tl, :], QP[d], AF.Identity,
                            scale=float(coef[d]), accum_out=tgt,
                        )
                    else:
                        nc.vector.tensor_scalar(
                            out=js[:tl, :], in0=QP[d], scalar1=float(coef[d]),
                            scalar2=0.0, op0=OP.mult, op1=OP.add,
                            accum_out=tgt,
                        )

                for d in range(1, D + 1):
                    if d in raw_ds:
                        continue
                    den_accum(d)

                # scale raw den moments by coefficients (tiny TT)
                if ncden:
                    nc.vector.tensor_mul(
                        A2d[:tl, 0:ncden], SmD[:tl, 0:ncden], cden[:tl, 0:ncden]
                    )

                # ---- num moments m_d (coef folded into op scalar/scale)
                for d in range(1, D + 1):
                    mode = CFG["m"][d - 1]
                    if mode == "ttr":
                        sd = scrp.tile([128, H], FW, tag="scr")
                        nc.vector.tensor_tensor_reduce(
                            out=sd[:tl, :], in0=QP[d], in1=X[:tl, :],
                            scale=float(coef[d]), scalar=0.0,
                            op0=OP.mult, op1=OP.add,
                            accum_out=A2n[:tl, d : d + 1],
                        )
                    elif mode == "stt":
                        sd = scrp.tile([128, H], FW, tag="scr")
                        nc.vector.scalar_tensor_tensor(
                            out=sd[:tl, :], in0=QP[d], scalar=float(coef[d]),
                            in1=X[:tl, :], op0=OP.mult, op1=OP.mult,
                            accum_out=A2n[:tl, d : d + 1],
                        )
                    else:
                        vd = scrp.tile([128, H], FW, tag=f"v{d}")
                        nc.gpsimd.tensor_mul(vd[:tl, :], QP[d], X[:tl, :])
                        jd = scrp.tile([128, H], FW, tag="scr")
                        if mode == "pool_act":
                            nc.scalar.activation(
                                jd[:tl, :], vd[:tl, :], AF.Identity,
                                scale=float(coef[d]),
                                accum_out=A2n[:tl, d : d + 1],
                            )
                        else:
                            nc.vector.tensor_scalar(
                                out=jd[:tl, :], in0=vd[:tl, :],
                                scalar1=float(coef[d]), scalar2=0.0,
                                op0=OP.mult, op1=OP.add,
                                accum_out=A2n[:tl, d : d + 1],
                            )

                return {
                    "X": X, "KP": KP, "A2n": A2n, "A2d": A2d,
                    "dcol": dcol, "aN0": aN0,
                }

        def tail(bi, st):
            t0, tl = BLOCKS[bi]
            if True:
                X, KP, A2n, A2d = st["X"], st["KP"], st["A2n"], st["A2d"]
                aN0 = st["aN0"]
                K = KP[1]

                # ---- chains in the power basis (ascending)
                def chain_horner(a, tag, last_f32=False, bias_end=None):
                    # u = a(D)*k; u = (u + a(d))*k for d=D-1..1 [; u += bias_end]
                    u = work.tile([128, H], FW, tag=f"u{tag}")
                    nc.vector.tensor_scalar(
                        out=u[:tl, :], in0=K, scalar1=a(D), scalar2=None,
                        op0=OP.mult,
                    )
                    cur = u
                    for d in range(D - 1, 0, -1):
                        last = d == 1 and bias_end is None
                        odt = F32 if (last and last_f32) else FW
                        nxt_t = work.tile([128, H], odt, tag=f"u{tag}{d}")
                        nc.vector.scalar_tensor_tensor(
                            out=nxt_t[:tl, :], in0=cur[:tl, :], scalar=a(d),
                            in1=K, op0=OP.add, op1=OP.mult,
                        )
                        cur = nxt_t
                    if bias_end is not None:
                        fin = work.tile(
                            [128, H], F32 if last_f32 else FW, tag=f"u{tag}f"
                        )
                        nc.vector.tensor_scalar(
                            out=fin[:tl, :], in0=cur[:tl, :], scalar1=bias_end,
                            scalar2=None, op0=OP.add,
                        )
                        cur = fin
                    return cur

                def chain(modes, a, tag, bias0=None, last_f32=False):
                    u = work.tile([128, H], FW, tag=f"u{tag}")
                    if modes[0] == "act":
                        nc.scalar.activation(
                            u[:tl, :], K, AF.Identity, scale=a(1),
                            bias=0.0 if bias0 is None else bias0,
                        )
                    elif bias0 is None:
                        nc.vector.tensor_scalar(
                            out=u[:tl, :], in0=K, scalar1=a(1),
                            scalar2=None, op0=OP.mult,
                        )
                    else:
                        nc.vector.tensor_scalar(
                            out=u[:tl, :], in0=K, scalar1=a(1),
                            scalar2=bias0, op0=OP.mult, op1=OP.add,
                        )
                    cur = u
                    for d in range(2, D + 1):
                        last = d == D
                        odt = F32 if (last and last_f32) else FW
                        mode = modes[d - 1]
                        nxt_t = work.tile([128, H], odt, tag=f"u{tag}{d}")
                        if mode == "stt":
                            nc.vector.scalar_tensor_tensor(
                                out=nxt_t[:tl, :], in0=KP[d], scalar=a(d),
                                in1=cur[:tl, :], op0=OP.mult, op1=OP.add,
                            )
                        else:
                            td = scrp.tile([128, H], FW, tag="scr")
                            nc.vector.tensor_scalar(
                                out=td[:tl, :], in0=KP[d], scalar1=a(d),
                                scalar2=None, op0=OP.mult,
                            )
                            add_eng = nc.gpsimd if mode == "ts_pool" else nc.vector
                            add_eng.tensor_add(
                                nxt_t[:tl, :], td[:tl, :], cur[:tl, :]
                            )
                        cur = nxt_t
                    return cur

                aDf = lambda d: A2d[:tl, dcol[d] : dcol[d] + 1]
                aNf = lambda d: A2n[:tl, d : d + 1]
                if CFG["uD_form"] == "horner":
                    uD = chain_horner(aDf, "d", last_f32=True, bias_end=aD0)
                else:
                    uD = chain(CFG["uD"], aDf, "d", bias0=aD0, last_f32=True)
                if CFG["uN_form"] == "horner":
                    uN = chain_horner(aNf, "n")
                else:
                    uN = chain(CFG["uN"], aNf, "n")

                # ---- out = (uN + aN0) * (1/den)
                rD = work.tile([128, H], F32, tag="rD")
                nc.vector.reciprocal_approx_fast(rD[:tl, :], uD[:tl, :])
                O = io.tile([128, H], F32, tag="O")
                if CFG["final"] == "stt":
                    nc.vector.scalar_tensor_tensor(
                        out=O[:tl, :], in0=uN[:tl, :], scalar=aN0,
                        in1=rD[:tl, :], op0=OP.add, op1=OP.mult,
                    )
                else:
                    tf = scrp.tile([128, H], FW, tag="scr")
                    nc.vector.tensor_scalar(
                        out=tf[:tl, :], in0=uN[:tl, :], scalar1=aN0,
                        scalar2=None, op0=OP.add,
                    )
                    nc.gpsimd.tensor_mul(O[:tl, :], tf[:tl, :], rD[:tl, :])
                out_eng.dma_start(out=out[t0 : t0 + tl, :], in_=O[:tl, :])

        def body():
            if CFG["interleave"]:
                s0 = head(0)
                s1 = head(1)
                tail(0, s0)
                tail(1, s1)
            else:
                for bi in range(len(BLOCKS)):
                    tail(bi, head(bi))

        if reps == 1:
            body()
        else:
            with tc.For_i(0, reps, 1):
                body()

    nc.compile()
    return nc


_NCS = {}


def _get_nc(with_bias: bool = True):
    if with_bias not in _NCS:
        _NCS[with_bias] = build_kernel(with_bias=with_bias)
    return _NCS[with_bias]


def _make_in_maps(x, W0, b0, W1, b1):
    coef = COEFS[D]
    raw_ds = [1] + ([2] if CFG["q2"] == "act" and CFG["s2"] == "free" else [])
    ncden = len(raw_ds)
    npw = np.float16 if CFG["dtype"] == "f16" else np.float32
    xf = np.ascontiguousarray(np.asarray(x, np.float32).reshape(T, H))
    W0 = np.asarray(W0, np.float32).astype(npw)
    W1 = np.asarray(W1, np.float32).astype(npw)
    biasQ = np.zeros((128, H), npw)
    biasQ[0, :] = np.asarray(b1, np.float32).astype(npw)
    biasK = np.zeros((128, H), npw)
    biasK[0, :] = np.asarray(b0, np.float32).astype(npw)
    cdn = np.tile(
        np.array([coef[d] for d in raw_ds], np.float32).reshape(1, ncden),
        (128, 1),
    ).astype(np.float32)
    wcat = np.ascontiguousarray(
        np.concatenate(
            [W1[:128, :], W1[128:, :], biasQ, W0[:128, :], W0[128:, :], biasK],
            axis=1,
        )
    )  # [128, 6H] fp16
    maps = []
    for c in range(NCORES):
        sh = np.ascontiguousarray(xf[c * TC : (c + 1) * TC]).astype(npw)
        xst = np.ascontiguousarray(
            np.transpose(sh.reshape(TC, 2, 128), (2, 1, 0))
        )
        maps.append({"xs": sh, "xst": xst, "wcat": wcat, "cdn": cdn})
    return maps


def _ensure_axon():
    try:
        import jax
        if not any(d.platform == "axon" for d in jax.devices()):
            jax.config.update("jax_platforms", "axon,cpu")
    except Exception:
        pass


def _run(x, W0, b0, W1, b1, trace=False, **kw):
    _ensure_axon()
    with_bias = bool(
        np.any(np.asarray(b0, np.float32)) or np.any(np.asarray(b1, np.float32))
    )
    res = run_bass_kernel_spmd(
        _get_nc(with_bias), _make_in_maps(x, W0, b0, W1, b1),
        list(range(NCORES)), trace=trace, **kw,
    )
    outs = [res.results[c]["out"] for c in range(NCORES)]
    full = np.concatenate(outs, axis=0).reshape(B, S, M, H).astype(np.float32)
    return full, res


def kernel(x, W0, b0, W1, b1):
    full, _ = _run(x, W0, b0, W1, b1, trace=False)
    return full


# revision 36
# speedup vs baseline: 1.8878x; 1.0789x over previous
"""Trainium2 Bass kernel for per-token outer-product softmax attention.

Reference computation (per token t of 1600, H=256):
    k = tanh(x W0 + b0);  q = tanh(x W1 + b1)
    scores[i,j] = k[i]*q[j];  attn = softmax_j(scores);  out = attn @ x

Key algebra: k,q are tanh outputs so k[i]*q[j] in (-1,1). On [-1,1],
exp(s) is approximated well below the 2e-2 output tolerance by a
low-degree minimax polynomial P(s) = sum_d c_d s^d, and P(k_i q_j) =
sum_d c_d k_i^d q_j^d is SEPARABLE. Softmax numerator/denominator become
per-token moments:
    num_i = sum_d (c_d sum_j q_j^d x_j) k_i^d
    den_i = sum_d (c_d sum_j q_j^d)     k_i^d
so the 256x256 scores tensor is never materialized. Moments come free as
accum_out of the product ops (coefficients folded into the op scalars,
m0 via a tiny PE ones-matmul); both k-polynomials are evaluated as
Horner chains of fused DVE scalar_tensor_tensor steps (no k-power tiles
needed). Working dtype is fp16 (output fp32): end-to-end rel-L2 err
~2.9e-3 at D=3, tolerance 2e-2.

Sharding: pure data parallel over tokens, 200 tokens/core x 8 cores;
weights replicated.
"""

import numpy as np
from contextlib import ExitStack

import concourse.bass as bass
import concourse.bacc as bacc
import concourse.tile as tile
from concourse import mybir
from concourse.bass_utils import run_bass_kernel_spmd

F32 = mybir.dt.float32
F16 = mybir.dt.float16
AF = mybir.ActivationFunctionType
OP = mybir.AluOpType

B, S, M, H = 4, 10, 40, 256
T = B * S * M            # 1600 tokens
NCORES = 8
TC = T // NCORES         # 200 tokens per core
BLOCKS = [(0, 128), (128, TC - 128)]

# Minimax-relative-error coefficients (monomial basis) of exp on [-1,1].
# Poly max rel err: D=3 -> 5.0e-3, D=4 -> 5.0e-4; end-to-end output
# rel-L2 err: D=3 fp16 ~3e-3, D=4 fp32 2.9e-4 (tolerance 2e-2).
COEFS = {
    3: [0.99650635, 1.0107962638, 0.5388581246, 0.1585305384],
    4: [0.9996280079, 0.9979377479, 0.5028966853, 0.1764876527,
        0.0399652955],
}

D = 3

# Placement/config knobs (tuned via CoreSim + HW loop benchmarks).
CFG = {
    "dtype": "f16",           # working dtype for powers/chains/products
    "q2": "act",              # act (Square, free s2 accum) | pool | dve
    "k2": "pool",             # act | pool | dve
    "q3": "pool",             # pool | dve
    "k3": "pool",
    "s1": "dve",              # free (tanh accum + scale TT) | dve (TS-acc)
    "s2": "dve",              # free (q2==act) | dve (TS-acc) | act (Id-acc)
    "s3": "dve",              # dve | act
    "m0": "pe",               # pe (ones-matmul) | dve | act
    # m1..mD: "ttr" (DVE tensor_tensor_reduce, 1 op) | "stt" (DVE fused)
    #         | "pool_dve" (Pool product + DVE TS-acc)
    #         | "pool_act" (Pool product + ACT Id-acc)
    # NOTE: "ttr" (tensor_tensor_reduce) crashes on HW — do not use.
    "m": ["stt", "stt", "stt", "stt"][: D],
    # chain form: "power" (ascending power basis, needs K2/K3) or
    # "horner" (descending, all-DVE TS+STT, no K powers needed)
    "uN_form": "horner",
    "uD_form": "horner",
    # power-form steps 1..D: step1: "dve" (TS) | "act" (Id scale+bias);
    # steps>=2: "stt" (DVE fused) | "ts_pool" (DVE TS + Pool add)
    #           | "ts_dve" (DVE TS + DVE add)
    "uN": ["dve", "stt", "stt", "stt"][: D],
    "uD": ["dve", "stt", "stt", "stt"][: D],
    "final": "stt",           # stt (DVE) | ts_pool (DVE TS + Pool mult)
    "interleave": True,       # emit b0 head, b1 head, b0 tail, b1 tail
    "w_dma": "gpsimd",        # queue for weight DMAs: sync | scalar | gpsimd
    "x_dma": "sync",
    "out_dma": "sync",
    "io_bufs": 4,
    "work_bufs": 4,
    "pows_bufs": 4,
    "mom_bufs": 4,
    "ps_bufs": 3,
    "scrp_bufs": 16,
}


def build_kernel(reps: int = 1, with_bias: bool = True) -> bass.Bass:
    coef = COEFS[D]
    FW = F16 if CFG["dtype"] == "f16" else F32
    # raw den-moment columns needing the cden scale TT
    raw_ds = [d for d, k in [(1, "s1"), (2, "s2")] if CFG[k] == "free"]
    ncden = len(raw_ds)
    # col layout in A2d: raw cols first, then direct-scaled cols
    dcol = {}
    for i, d in enumerate(raw_ds):
        dcol[d] = i
    nxt = ncden
    for d in range(1, D + 1):
        if d not in dcol:
            dcol[d] = nxt
            nxt += 1

    WW = 2 * H + H + 2 * H + H   # [W1lo|W1hi|biasQ || W0lo|W0hi|biasK] fp16
    nc = bacc.Bacc("TRN2", target_bir_lowering=False, debug=False)
    xs = nc.declare_dram_parameter("xs", [TC, H], FW, isOutput=False)
    xst = nc.declare_dram_parameter("xst", [128, 2, TC], FW, isOutput=False)
    wcat = nc.declare_dram_parameter("wcat", [128, WW], FW, isOutput=False)
    cdn = nc.declare_dram_parameter("cdn", [128, max(ncden, 1)], F32, isOutput=False)
    out = nc.declare_dram_parameter("out", [TC, H], F32, isOutput=True)

    with tile.TileContext(nc) as tc, ExitStack() as ctx:
        consts = ctx.enter_context(tc.tile_pool(name="consts", bufs=1))
        io = ctx.enter_context(tc.tile_pool(name="io", bufs=CFG["io_bufs"]))
        work = ctx.enter_context(tc.tile_pool(name="work", bufs=CFG["work_bufs"]))
        pows = ctx.enter_context(tc.tile_pool(name="pows", bufs=CFG["pows_bufs"]))
        scrp = ctx.enter_context(tc.tile_pool(name="scrp", bufs=CFG["scrp_bufs"]))
        mom = ctx.enter_context(tc.tile_pool(name="mom", bufs=CFG["mom_bufs"]))
        psKQ = ctx.enter_context(
            tc.tile_pool(name="psKQ", bufs=CFG["ps_bufs"], space="PSUM")
        )

        eng = {"dve": nc.vector, "pool": nc.gpsimd, "act": nc.scalar}
        x_eng = getattr(nc, CFG["x_dma"])
        out_eng = getattr(nc, CFG["out_dma"])

        ones1 = consts.tile([1, 128], FW)
        if with_bias:
            nc.gpsimd.memset(ones1, 1.0)
        if CFG["m0"] == "pe":
            c0col = consts.tile([128, 1], FW)
            nc.gpsimd.memset(c0col, float(coef[0]))
            psM = ctx.enter_context(tc.tile_pool(name="psM", bufs=2, space="PSUM"))
        Xs = []
        XTs = []
        for t0, tl in BLOCKS:
            X = io.tile([128, H], FW, tag=f"X{t0}")
            x_eng.dma_start(out=X[:tl, :], in_=xs[t0 : t0 + tl, :])
            Xs.append(X)
            xT = io.tile([128, 2, 128], FW, tag=f"XT{t0}")
            nc.gpsimd.dma_start(out=xT[:, :, :tl], in_=xst[:, :, t0 : t0 + tl])
            XTs.append(xT)
        w_eng = getattr(nc, CFG["w_dma"])
        wallQ = consts.tile([128, 3 * H], FW)
        w_eng.dma_start(out=wallQ, in_=wcat[:, 0 : 3 * H])
        wallK = consts.tile([128, 3 * H], FW)
        w_eng.dma_start(out=wallK, in_=wcat[:, 3 * H : 6 * H])
        cden = consts.tile([128, max(ncden, 1)], F32)
        w_eng.dma_start(out=cden, in_=cdn[:, :])
        bsbQ = wallQ[0:1, 2 * H : 3 * H]
        bsbK = wallK[0:1, 2 * H : 3 * H]
        aD0 = float(coef[0]) * float(H)

        def head(bi):
            t0, tl = BLOCKS[bi]
            if True:
                X = Xs[bi]
                xT = XTs[bi]

                # ---- matmuls: queries first (moments only need Q and X)
                psQ = psKQ.tile([128, H], F32, tag="psQ")
                if with_bias:
                    nc.tensor.matmul(
                        psQ[:tl, :], ones1[:, :tl], bsbQ, start=True, stop=False
                    )
                nc.tensor.matmul(
                    psQ[:tl, :], xT[:, 0, :tl], wallQ[:, 0:256],
                    start=not with_bias, stop=False,
                )
                nc.tensor.matmul(
                    psQ[:tl, :], xT[:, 1, :tl], wallQ[:, 256:512],
                    start=False, stop=True,
                )
                psK = psKQ.tile([128, H], F32, tag="psK")
                if with_bias:
                    nc.tensor.matmul(
                        psK[:tl, :], ones1[:, :tl], bsbK, start=True, stop=False
                    )
                nc.tensor.matmul(
                    psK[:tl, :], xT[:, 0, :tl], wallK[:, 0:256],
                    start=not with_bias, stop=False,
                )
                nc.tensor.matmul(
                    psK[:tl, :], xT[:, 1, :tl], wallK[:, 256:512],
                    start=False, stop=True,
                )

                # A2n[:, d] = c_d * m_d ; A2d[:, dcol[d]] = c_d * s_d
                A2n = mom.tile([128, D + 1], F32, tag="A2n")
                SmD = mom.tile([128, max(ncden, 1)], F32, tag="SmD")
                A2d = mom.tile([128, D], F32, tag="A2d")

                # ---- aN0 = c0 * sum_j x
                if CFG["m0"] == "pe":
                    psM0 = psM.tile([128, 1], F32, tag="psM0")
                    nc.tensor.matmul(
                        psM0[:tl, :], xT[:, 0, :tl], c0col,
                        start=True, stop=False,
                    )
                    nc.tensor.matmul(
                        psM0[:tl, :], xT[:, 1, :tl], c0col,
                        start=False, stop=True,
                    )
                    aN0 = psM0[:tl, 0:1]
                else:
                    j0 = scrp.tile([128, H], FW, tag="scr")
                    if CFG["m0"] == "act":
                        nc.scalar.activation(
                            j0[:tl, :], X[:tl, :], AF.Identity,
                            scale=float(coef[0]), accum_out=A2n[:tl, 0:1],
                        )
                    else:
                        nc.vector.tensor_scalar(
                            out=j0[:tl, :], in0=X[:tl, :], scalar1=float(coef[0]),
                            scalar2=0.0, op0=OP.mult, op1=OP.add,
                            accum_out=A2n[:tl, 0:1],
                        )
                    aN0 = A2n[:tl, 0:1]

                # ---- tanh (accum gives raw s1 when s1 == "free")
                Qt = work.tile([128, H], FW, tag="Qt")
                kw1 = (
                    {"accum_out": SmD[:tl, dcol[1] : dcol[1] + 1]}
                    if CFG["s1"] == "free" else {}
                )
                nc.scalar.activation(Qt[:tl, :], psQ[:tl, :], AF.Tanh, **kw1)
                Q = Qt[:tl, :]
                Kt = work.tile([128, H], FW, tag="Kt")
                nc.scalar.activation(Kt[:tl, :], psK[:tl, :], AF.Tanh)
                K = Kt[:tl, :]

                # ---- powers
                Q2 = pows.tile([128, H], FW, tag="Q2")
                if CFG["q2"] == "act":
                    kw = (
                        {"accum_out": SmD[:tl, dcol[2] : dcol[2] + 1]}
                        if CFG["s2"] == "free" else {}
                    )
                    nc.scalar.activation(Q2[:tl, :], Q, AF.Square, **kw)
                else:
                    eng[CFG["q2"]].tensor_mul(Q2[:tl, :], Q, Q)
                need_kp = "power" in (CFG["uN_form"], CFG["uD_form"])
                KP = {1: K}
                if need_kp:
                    K2 = pows.tile([128, H], FW, tag="K2")
                    if CFG["k2"] == "act":
                        nc.scalar.activation(K2[:tl, :], K, AF.Square)
                    else:
                        eng[CFG["k2"]].tensor_mul(K2[:tl, :], K, K)
                    KP[2] = K2[:tl, :]
                Q3 = pows.tile([128, H], FW, tag="Q3")
                eng[CFG["q3"]].tensor_mul(Q3[:tl, :], Q2[:tl, :], Q)
                if need_kp:
                    K3 = pows.tile([128, H], FW, tag="K3")
                    eng[CFG["k3"]].tensor_mul(K3[:tl, :], K2[:tl, :], K)
                    KP[3] = K3[:tl, :]
                QP = {1: Q, 2: Q2[:tl, :], 3: Q3[:tl, :]}
                if D >= 4:
                    Q4 = pows.tile([128, H], FW, tag="Q4")
                    nc.scalar.activation(Q4[:tl, :], Q2[:tl, :], AF.Square)
                    K4 = pows.tile([128, H], FW, tag="K4")
                    nc.scalar.activation(K4[:tl, :], K2[:tl, :], AF.Square)
                    QP[4] = Q4[:tl, :]
                    KP[4] = K4[:tl, :]

                # ---- scaled den moments s_d -> A2d (direct for non-raw)
                def den_accum(d):
                    js = scrp.tile([128, H], FW, tag="scr")
                    tgt = A2d[:tl, dcol[d] : dcol[d] + 1]
                    mode = CFG["s" + str(min(d, 3))]
                    if mode == "act":
                        nc.scalar.activation(
                            js[:tl, :], QP[d], AF.Identity,
                            scale=float(coef[d]), accum_out=tgt,
                        )
                    else:
                        nc.vector.tensor_scalar(
                            out=js[:tl, :], in0=QP[d], scalar1=float(coef[d]),
                            scalar2=0.0, op0=OP.mult, op1=OP.add,
                            accum_out=tgt,
                        )

                for d in range(1, D + 1):
                    if d in raw_ds:
                        continue
                    den_accum(d)

                # scale raw den moments by coefficients (tiny TT)
                if ncden:
                    nc.vector.tensor_mul(
                        A2d[:tl, 0:ncden], SmD[:tl, 0:ncden], cden[:tl, 0:ncden]
                    )

                # ---- num moments m_d (coef folded into op scalar/scale)
                for d in range(1, D + 1):
                    mode = CFG["m"][d - 1]
                    if mode == "ttr":
                        sd = scrp.tile([128, H], FW, tag="scr")
                        nc.vector.tensor_tensor_reduce(
                            out=sd[:tl, :], in0=QP[d], in1=X[:tl, :],
                            scale=float(coef[d]), scalar=0.0,
                            op0=OP.mult, op1=OP.add,
                            accum_out=A2n[:tl, d : d + 1],
                        )
                    elif mode == "stt":
                        sd = scrp.tile([128, H], FW, tag="scr")
                        nc.vector.scalar_tensor_tensor(
                            out=sd[:tl, :], in0=QP[d], scalar=float(coef[d]),
                            in1=X[:tl, :], op0=OP.mult, op1=OP.mult,
                            accum_out=A2n[:tl, d : d + 1],
                        )
                    else:
                        vd = scrp.tile([128, H], FW, tag=f"v{d}")
                        nc.gpsimd.tensor_mul(vd[:tl, :], QP[d], X[:tl, :])
                        jd = scrp.tile([128, H], FW, tag="scr")
                        if mode == "pool_act":
                            nc.scalar.activation(
                                jd[:tl, :], vd[:tl, :], AF.Identity,
                                scale=float(coef[d]),
                                accum_out=A2n[:tl, d : d + 1],
                            )
                        else:
                            nc.vector.tensor_scalar(
                                out=jd[:tl, :], in0=vd[:tl, :],
                                scalar1=float(coef[d]), scalar2=0.0,
                                op0=OP.mult, op1=OP.add,
                                accum_out=A2n[:tl, d : d + 1],
                            )

                return {
                    "X": X, "KP": KP, "A2n": A2n, "A2d": A2d,
                    "dcol": dcol, "aN0": aN0,
                }

        def tail(bi, st):
            t0, tl = BLOCKS[bi]
            if True:
                X, KP, A2n, A2d = st["X"], st["KP"], st["A2n"], st["A2d"]
                aN0 = st["aN0"]
                K = KP[1]

                # ---- chains in the power basis (ascending)
                def chain_horner(a, tag, last_f32=False, bias_end=None):
                    # u = a(D)*k; u = (u + a(d))*k for d=D-1..1 [; u += bias_end]
                    u = work.tile([128, H], FW, tag=f"u{tag}")
                    nc.vector.tensor_scalar(
                        out=u[:tl, :], in0=K, scalar1=a(D), scalar2=None,
                        op0=OP.mult,
                    )
                    cur = u
                    for d in range(D - 1, 0, -1):
                        last = d == 1 and bias_end is None
                        odt = F32 if (last and last_f32) else FW
                        nxt_t = work.tile([128, H], odt, tag=f"u{tag}{d}")
                        nc.vector.scalar_tensor_tensor(
                            out=nxt_t[:tl, :], in0=cur[:tl, :], scalar=a(d),
                            in1=K, op0=OP.add, op1=OP.mult,
                        )
                        cur = nxt_t
                    if bias_end is not None:
                        fin = work.tile(
                            [128, H], F32 if last_f32 else FW, tag=f"u{tag}f"
                        )
                        nc.vector.tensor_scalar(
                            out=fin[:tl, :], in0=cur[:tl, :], scalar1=bias_end,
                            scalar2=None, op0=OP.add,
                        )
                        cur = fin
                    return cur

                def chain(modes, a, tag, bias0=None, last_f32=False):
                    u = work.tile([128, H], FW, tag=f"u{tag}")
                    if modes[0] == "act":
                        nc.scalar.activation(
                            u[:tl, :], K, AF.Identity, scale=a(1),
                            bias=0.0 if bias0 is None else bias0,
                        )
                    elif bias0 is None:
                        nc.vector.tensor_scalar(
                            out=u[:tl, :], in0=K, scalar1=a(1),
                            scalar2=None, op0=OP.mult,
                        )
                    else:
                        nc.vector.tensor_scalar(
                            out=u[:tl, :], in0=K, scalar1=a(1),
                            scalar2=bias0, op0=OP.mult, op1=OP.add,
                        )
                    cur = u
                    for d in range(2, D + 1):
                        last = d == D
                        odt = F32 if (last and last_f32) else FW
                        mode = modes[d - 1]
                        nxt_t = work.tile([128, H], odt, tag=f"u{tag}{d}")
                        if mode == "stt":
                            nc.vector.scalar_tensor_tensor(
                                out=nxt_t[:tl, :], in0=KP[d], scalar=a(d),
                                in1=cur[:tl, :], op0=OP.mult, op1=OP.add,
                            )
                        else:
                            td = scrp.tile([128, H], FW, tag="scr")
                            nc.vector.tensor_scalar(
                                out=td[:tl, :], in0=KP[d], scalar1=a(d),
                                scalar2=None, op0=OP.mult,
                            )
                            add_eng = nc.gpsimd if mode == "ts_pool" else nc.vector
                            add_eng.tensor_add(
                                nxt_t[:tl, :], td[:tl, :], cur[:tl, :]
                            )
                        cur = nxt_t
                    return cur

                aDf = lambda d: A2d[:tl, dcol[d] : dcol[d] + 1]
                aNf = lambda d: A2n[:tl, d : d + 1]
                if CFG["uD_form"] == "horner":
                    uD = chain_horner(aDf, "d", last_f32=True, bias_end=aD0)
                else:
                    uD = chain(CFG["uD"], aDf, "d", bias0=aD0, last_f32=True)
                if CFG["uN_form"] == "horner":
                    uN = chain_horner(aNf, "n")
                else:
                    uN = chain(CFG["uN"], aNf, "n")

                # ---- out = (uN + aN0) * (1/den)
                rD = work.tile([128, H], F32, tag="rD")
                nc.vector.reciprocal_approx_fast(rD[:tl, :], uD[:tl, :])
                O = io.tile([128, H], F32, tag="O")
                if CFG["final"] == "stt":
                    nc.vector.scalar_tensor_tensor(
                        out=O[:tl, :], in0=uN[:tl, :], scalar=aN0,
                        in1=rD[:tl, :], op0=OP.add, op1=OP.mult,
                    )
                else:
                    tf = scrp.tile([128, H], FW, tag="scr")
                    nc.vector.tensor_scalar(
                        out=tf[:tl, :], in0=uN[:tl, :], scalar1=aN0,
                        scalar2=None, op0=OP.add,
                    )
                    nc.gpsimd.tensor_mul(O[:tl, :], tf[:tl, :], rD[:tl, :])
                out_eng.dma_start(out=out[t0 : t0 + tl, :], in_=O[:tl, :])

        def body():
            if CFG["interleave"]:
                s0 = head(0)
                s1 = head(1)
                tail(0, s0)
                tail(1, s1)
            else:
                for bi in range(len(BLOCKS)):
                    tail(bi, head(bi))

        if reps == 1:
            body()
        else:
            with tc.For_i(0, reps, 1):
                body()

    nc.compile()
    return nc


_NCS = {}


def _get_nc(with_bias: bool = True):
    if with_bias not in _NCS:
        _NCS[with_bias] = build_kernel(with_bias=with_bias)
    return _NCS[with_bias]


def _make_in_maps(x, W0, b0, W1, b1):
    coef = COEFS[D]
    raw_ds = [d for d, k in [(1, "s1"), (2, "s2")] if CFG[k] == "free"]
    ncden = len(raw_ds)
    npw = np.float16 if CFG["dtype"] == "f16" else np.float32
    xf = np.ascontiguousarray(np.asarray(x, np.float32).reshape(T, H))
    W0 = np.asarray(W0, np.float32).astype(npw)
    W1 = np.asarray(W1, np.float32).astype(npw)
    biasQ = np.zeros((128, H), npw)
    biasQ[0, :] = np.asarray(b1, np.float32).astype(npw)
    biasK = np.zeros((128, H), npw)
    biasK[0, :] = np.asarray(b0, np.float32).astype(npw)
    cdn = np.tile(
        np.array(
            [coef[d] for d in raw_ds] or [0.0], np.float32
        ).reshape(1, max(ncden, 1)),
        (128, 1),
    ).astype(np.float32)
    wcat = np.ascontiguousarray(
        np.concatenate(
            [W1[:128, :], W1[128:, :], biasQ, W0[:128, :], W0[128:, :], biasK],
            axis=1,
        )
    )  # [128, 6H] fp16
    maps = []
    for c in range(NCORES):
        sh = np.ascontiguousarray(xf[c * TC : (c + 1) * TC]).astype(npw)
        xst = np.ascontiguousarray(
            np.transpose(sh.reshape(TC, 2, 128), (2, 1, 0))
        )
        maps.append({"xs": sh, "xst": xst, "wcat": wcat, "cdn": cdn})
    return maps


def _ensure_axon():
    try:
        import jax
        if not any(d.platform == "axon" for d in jax.devices()):
            jax.config.update("jax_platforms", "axon,cpu")
    except Exception:
        pass


def _run(x, W0, b0, W1, b1, trace=False, **kw):
    _ensure_axon()
    with_bias = bool(
        np.any(np.asarray(b0, np.float32)) or np.any(np.asarray(b1, np.float32))
    )
    res = run_bass_kernel_spmd(
        _get_nc(with_bias), _make_in_maps(x, W0, b0, W1, b1),
        list(range(NCORES)), trace=trace, **kw,
    )
    outs = [res.results[c]["out"] for c in range(NCORES)]
    full = np.concatenate(outs, axis=0).reshape(B, S, M, H).astype(np.float32)
    return full, res


def kernel(x, W0, b0, W1, b1):
    full, _ = _run(x, W0, b0, W1, b1, trace=False)
    return full
